# revision 1
# baseline (speedup 1.0000x reference)
"""Trainium2 Bass kernel for nn_AutoregressiveSelfAttention.

Sharding (8 cores): batch (2-way) x head-group (4-way tensor parallel).
Core c: batch c//4, heads [4*(c%4), 4*(c%4)+4).
Per-core: fp16 matmuls throughout (QKV proj, scores, P@V, out-proj),
fp32 softmax statistics, fp32 partial output; host sums the 4 head-group
partials per batch (the row-parallel all-reduce) and transposes back.

Softmax without transposes: pass1 computes scores [sq, sk] only to get the
causal row-max M (fused mask+max on DVE); pass2 recomputes scores
transposed with the max-subtraction folded in as a rank-1 matmul term
([kT;1].T @ [qT;-M]), exps on ACT into fp16, and the ctx matmul against
[v|1] accumulates both ctx and the softmax denominator in one PSUM tile.
Emission is interleaved per (seq-chunk, head-pair) so the DVE-bound pass1
and ACT-bound pass2 overlap.
"""
import sys
sys.path.insert(0, "/opt/trn_rl_repo")

import math
import numpy as np

B, S, E, H, D = 2, 2048, 1024, 16, 64
NCORES = 8
HG = 4                  # head-group shards
HPC = H // HG           # 4 heads per core
OC = HPC * D            # 256 per-core projection width
P = 128
NK = E // P             # 8 contraction tiles for projections
NT = S // P             # 16 seq tiles of 128
NJ = S // 512           # 4 seq chunks of 512

_CACHE = {}


def _build():
    import concourse.bacc as bacc
    import concourse.mybir as mybir
    import concourse.tile as tile
    from concourse.masks import make_identity, make_causal_mask

    dt = mybir.dt
    f32, f16 = dt.float32, dt.float16
    AX = mybir.AxisListType.X
    ALU = mybir.AluOpType

    nc = bacc.Bacc(None, target_bir_lowering=False, debug=False)
    with tile.TileContext(nc) as tc:
        with tc.tile_pool(name="dram", bufs=1, space="DRAM") as dram, \
             tc.tile_pool(name="persist", bufs=1) as pers, \
             tc.tile_pool(name="stream", bufs=4) as strm, \
             tc.tile_pool(name="tmp", bufs=4) as tmp, \
             tc.tile_pool(name="ps", bufs=1, space="PSUM") as ps:

            # ---- DRAM I/O ----
            xt = dram.tile([E, S], f16, kind="ExternalInput", name="xt", uniquify=False)
            wq = dram.tile([E, OC], f16, kind="ExternalInput", name="wq", uniquify=False)
            wk = dram.tile([E, OC], f16, kind="ExternalInput", name="wk", uniquify=False)
            wv = dram.tile([E, OC], f16, kind="ExternalInput", name="wv", uniquify=False)
            wo = dram.tile([OC, E], f16, kind="ExternalInput", name="wo", uniquify=False)
            outT = dram.tile([E, S], f32, kind="ExternalOutput", name="outT", uniquify=False)

            # ---- persistent SBUF ----
            xt_sb = pers.tile([P, NK, S], f16)
            wq_sb = pers.tile([P, NK, OC], f16)
            wk_sb = pers.tile([P, NK, OC], f16)
            wv_sb = pers.tile([P, NK, OC], f16)
            wo_sb = pers.tile([P, 2, E], f16)
            qp_sb = pers.tile([P, 2, S], f16)       # qT, head-pair stacked
            kp_sb = pers.tile([P, 2, S], f16)       # kT, head-pair stacked
            qaug = pers.tile([65, HPC, S], f16)     # [qT_h ; -M_h] per head
            kaug = pers.tile([65, HPC, S], f16)     # [kT_h ; ones] per head
            vv0 = pers.tile([P, NT, 2, 65], f16)    # heads 0,2: [v(0:64) | ones(64)]
            vv1 = pers.tile([P, NT, 2, P], f16)     # heads 1,3: [ones | 0*63 | v(64:128)]
            ctxn = pers.tile([P, 2, S], f16)        # normalized ctx, pair stacked
            m2 = pers.tile([P, 2, 32], f32)         # rowmax cols per pair (hh*16+t)
            ident = pers.tile([P, P], f32)
            ident16 = pers.tile([P, P], f16)
            cmask16 = pers.tile([P, P], f16)        # 0 / -30000 causal block

            # ---- input DMAs ----
            xt_v = xt[:].rearrange("(k p) s -> k p s", p=P)
            wq_v = wq[:].rearrange("(k p) o -> k p o", p=P)
            wk_v = wk[:].rearrange("(k p) o -> k p o", p=P)
            wv_v = wv[:].rearrange("(k p) o -> k p o", p=P)
            wo_v = wo[:].rearrange("(k p) e -> k p e", p=P)
            outT_v = outT[:].rearrange("(o p) s -> o p s", p=P)
            for k in range(NK):
                nc.sync.dma_start(out=xt_sb[:, k, :], in_=xt_v[k])
                nc.sync.dma_start(out=wq_sb[:, k, :], in_=wq_v[k])
                nc.sync.dma_start(out=wk_sb[:, k, :], in_=wk_v[k])
                nc.sync.dma_start(out=wv_sb[:, k, :], in_=wv_v[k])
            for kt in range(2):
                nc.sync.dma_start(out=wo_sb[:, kt, :], in_=wo_v[kt])

            # ---- constants ----
            make_identity(nc, ident[:, :])
            make_identity(nc, ident16[:, :])
            make_causal_mask(nc, cmask16[:, :], mask_val=-30000.0)
            nc.gpsimd.memset(kaug[64:65, :, :], 1.0)
            nc.gpsimd.memset(vv0[:, :, :, 64:65], 1.0)
            nc.gpsimd.memset(vv1[:, :, :, 0:1], 1.0)
            nc.gpsimd.memset(vv1[:, :, :, 1:64], 0.0)

            # ---- q/k projections (transposed layout, pair-stacked) ----
            for dst, w_sb in ((qp_sb, wq_sb), (kp_sb, wk_sb)):
                for ot in range(2):
                    for j in range(NJ):
                        pp = ps.tile([P, 512], f32, tag="proj", bufs=2)
                        for k in range(NK):
                            nc.tensor.matmul(
                                pp[:, :],
                                w_sb[:, k, 128 * ot:128 * ot + 128],
                                xt_sb[:, k, 512 * j:512 * j + 512],
                                start=(k == 0), stop=(k == NK - 1))
                        nc.vector.tensor_copy(dst[:, ot, 512 * j:512 * j + 512],
                                              pp[:, :])

            # ---- augmented qT/kT copies (partition shift -> DMA) ----
            def emit_aug(j):
                for h in range(HPC):
                    pr, hh = divmod(h, 2)
                    sl = slice(512 * j, 512 * j + 512)
                    nc.sync.dma_start(out=qaug[0:64, h, sl],
                                      in_=qp_sb[64 * hh:64 * hh + 64, pr, sl])
                    nc.sync.dma_start(out=kaug[0:64, h, sl],
                                      in_=kp_sb[64 * hh:64 * hh + 64, pr, sl])

            def emit_vproj(st):
                pv = ps.tile([P, OC], f32, tag="proj", bufs=2)
                for k in range(NK):
                    nc.tensor.matmul(
                        pv[:, :], xt_sb[:, k, P * st:P * st + P], wv_sb[:, k, :],
                        start=(k == 0), stop=(k == NK - 1))
                pv4 = pv[:, :].rearrange("p (g x d) -> p g x d", g=2, x=2)
                nc.vector.tensor_copy(vv0[:, st, :, 0:64], pv4[:, :, 0, :])
                nc.vector.tensor_copy(vv1[:, st, :, 64:P], pv4[:, :, 1, :])

            def emit_pass1(pr, t):
                # scores [sq, sk] for one sq-tile, 2-head tile-packed; fused
                # (+causal mask) -> rowmax into m2 columns.
                ncols = (t + 1) * P
                nch = (ncols + 511) // 512
                m4a = tmp.tile([P, 4], f32, tag="m4a")
                m4b = tmp.tile([P, 4], f32, tag="m4b")
                for c in range(nch):
                    n = min(512, ncols - 512 * c)
                    sa = ps.tile([P, 512], f32, tag="s1", bufs=2)
                    sb_ = ps.tile([P, 512], f32, tag="s1", bufs=2)
                    last = c == nch - 1
                    nc.tensor.matmul(
                        sa[:, :n], qp_sb[0:64, pr, P * t:P * t + P],
                        kp_sb[0:64, pr, 512 * c:512 * c + n],
                        start=True, stop=not last, tile_position=(0, 0))
                    nc.tensor.matmul(
                        sb_[:, :n], qp_sb[64:P, pr, P * t:P * t + P],
                        kp_sb[64:P, pr, 512 * c:512 * c + n],
                        start=True, stop=not last, tile_position=(64, 0))
                    if last:
                        doff = n - P
                        nc.tensor.matmul(sa[:, doff:doff + P], ident16[:, :],
                                         cmask16[:, :], start=False, stop=True)
                        nc.tensor.matmul(sb_[:, doff:doff + P], ident16[:, :],
                                         cmask16[:, :], start=False, stop=True)
                    nc.vector.reduce_max(m4a[:, c:c + 1], sa[:, :n], axis=AX)
                    nc.vector.reduce_max(m4b[:, c:c + 1], sb_[:, :n], axis=AX)
                nc.vector.reduce_max(m2[:, pr, t:t + 1], m4a[:, 0:nch], axis=AX)
                nc.vector.reduce_max(m2[:, pr, 16 + t:16 + t + 1], m4b[:, 0:nch],
                                     axis=AX)

            def emit_mrow(pr, j):
                # -M for chunk j's four sq-tiles -> row 64 of qaug, per head.
                for hh in range(2):
                    mt_ps = ps.tile([4, P], f32, tag="s1", bufs=2)
                    nc.tensor.transpose(
                        mt_ps[:, :], m2[:, pr, 16 * hh + 4 * j:16 * hh + 4 * j + 4],
                        ident[:, :])
                    mt_t = tmp.tile([4, P], f16, tag="mt")
                    nc.vector.tensor_scalar_mul(mt_t[:, :], mt_ps[:, :], -1.0)
                    nc.sync.dma_start(
                        out=qaug[64:65, 2 * pr + hh,
                                 512 * j:512 * j + 512].rearrange(
                                     "q (t p) -> q t p", t=4),
                        in_=mt_t[:, :])

            def emit_pass2(h, j):
                # scoresT with folded -M, exp, causal zeroing, ctx+rowsum
                # accumulation, and normalization into ctxn.
                pr, hh = divmod(h, 2)
                ctxp = ps.tile([P, 512], f32, tag="ctx", bufs=2)
                nt_here = 4 * j + 4
                for t in range(nt_here):
                    if t < 4 * j:
                        qoff, n = 512 * j, 512
                    else:
                        qoff = P * t
                        n = 512 * j + 512 - P * t
                    s2p = ps.tile([P, 512], f32, tag="s2", bufs=2)
                    nc.tensor.matmul(
                        s2p[:, :n], kaug[:, h, P * t:P * t + P],
                        qaug[:, h, qoff:qoff + n], start=True, stop=True)
                    pt = strm.tile([P, 512], f16, tag="pt", bufs=6)
                    nc.scalar.activation(pt[:, :n], s2p[:, :n],
                                         mybir.ActivationFunctionType.Exp,
                                         scale=8.0)
                    if t >= 4 * j:
                        # zero strictly-upper block at the diagonal
                        nc.gpsimd.affine_select(
                            out=pt[:, 0:P], in_=pt[:, 0:P],
                            compare_op=ALU.is_ge, fill=0.0, base=0,
                            pattern=[[1, P]], channel_multiplier=-1)
                    lhsT = vv0[:, t, pr, :] if hh == 0 else vv1[:, t, pr, :]
                    nc.tensor.matmul(
                        ctxp[0:(65 if hh == 0 else P),
                             qoff - 512 * j:qoff - 512 * j + n],
                        lhsT, pt[:, :n],
                        start=(t == 0), stop=(t == nt_here - 1))
                # normalize: ctx / rowsum
                rsrow = 64 if hh == 0 else 0
                rr = tmp.tile([65, 512], f32, tag="rr")
                nc.vector.reciprocal(rr[rsrow:rsrow + 1, :],
                                     ctxp[rsrow:rsrow + 1, :])
                rb = tmp.tile([P, 512], f32, tag="rb")
                nc.sync.dma_start(
                    out=rb[64 * hh:64 * hh + 64, :],
                    in_=rr[rsrow:rsrow + 1, :].unsqueeze(1).broadcast_to(
                        (1, 64, 512)))
                nc.vector.tensor_mul(
                    ctxn[64 * hh:64 * hh + 64, pr, 512 * j:512 * j + 512],
                    ctxp[64 * hh:64 * hh + 64, :],
                    rb[64 * hh:64 * hh + 64, :])

            def emit_outproj(j):
                for oo in range(E // P):
                    po = ps.tile([P, 512], f32, tag="proj", bufs=2)
                    for kt in range(2):
                        nc.tensor.matmul(
                            po[:, :], wo_sb[:, kt, P * oo:P * oo + P],
                            ctxn[:, kt, 512 * j:512 * j + 512],
                            start=(kt == 0), stop=(kt == 1))
                    ob = strm.tile([P, 512], f32, tag="ob", bufs=3)
                    nc.scalar.copy(ob[:, :], po[:, :])
                    nc.sync.dma_start(out=outT_v[oo][:, 512 * j:512 * j + 512],
                                      in_=ob[:, :])

            # ---- interleaved attention pipeline ----
            for j in range(NJ):
                emit_aug(j)
                for st in range(4 * j, 4 * j + 4):
                    emit_vproj(st)
                for pr in range(2):
                    for t in range(4 * j, 4 * j + 4):
                        emit_pass1(pr, t)
                    emit_mrow(pr, j)
                    emit_pass2(2 * pr, j)
                    emit_pass2(2 * pr + 1, j)
                emit_outproj(j)

    nc.compile()
    return nc


def _get_nc():
    if "nc" not in _CACHE:
        _CACHE["nc"] = _build()
    return _CACHE["nc"]


def _make_cached_runner(nc):
    """Trace/compile the 8-core PJRT executable once; reuse on later calls.

    Mirrors concourse.bass2jax.run_bass_via_pjrt's multi-core branch, but
    keeps the jitted shard_map so repeat kernel() calls skip re-trace and
    re-lowering (the NEFF itself is already cached by neuronx_cc_hook).
    """
    import jax
    import jax.numpy as jnp
    from jax.sharding import Mesh, PartitionSpec
    from jax.experimental.shard_map import shard_map
    from concourse import bass2jax, mybir

    bass2jax.install_neuronx_cc_hook()
    partition_name = nc.partition_id_tensor.name if nc.partition_id_tensor else None
    in_names, out_names, out_avals = [], [], []
    for alloc in nc.m.functions[0].allocations:
        if not isinstance(alloc, mybir.MemoryLocationSet):
            continue
        name = alloc.memorylocations[0].name
        if alloc.kind == "ExternalInput":
            if name != partition_name:
                in_names.append(name)
        elif alloc.kind == "ExternalOutput":
            out_names.append(name)
            out_avals.append(jax.core.ShapedArray(
                tuple(alloc.tensor_shape), mybir.dt.np(alloc.dtype)))
    n_params = len(in_names)
    n_outs = len(out_avals)
    all_names = list(in_names) + list(out_names)
    if partition_name is not None:
        all_names.append(partition_name)

    def _body(*args):
        operands = list(args)
        if partition_name is not None:
            operands.append(bass2jax.partition_id_tensor())
        outs = bass2jax._bass_exec_p.bind(
            *operands,
            out_avals=tuple(out_avals),
            in_names=tuple(all_names),
            out_names=tuple(out_names),
            lowering_input_output_aliases=(),
            sim_require_finite=True,
            sim_require_nnan=True,
            nc=nc,
        )
        return tuple(outs)

    devices = jax.devices()[:NCORES]
    mesh = Mesh(np.asarray(devices), ("core",))
    in_specs = (PartitionSpec("core"),) * (n_params + n_outs)
    out_specs = (PartitionSpec("core"),) * n_outs
    sharded = jax.jit(
        shard_map(_body, mesh=mesh, in_specs=in_specs, out_specs=out_specs,
                  check_rep=False),
        donate_argnums=tuple(range(n_params, n_params + n_outs)),
        keep_unused=True)

    def run(in_maps):
        concat_in = [
            np.concatenate([np.asarray(in_maps[c][nm]) for c in range(NCORES)],
                           axis=0)
            for nm in in_names]
        concat_zeros = [
            np.zeros((NCORES * a.shape[0], *a.shape[1:]), a.dtype)
            for a in out_avals]
        out_arrs = sharded(*concat_in, *concat_zeros)
        return [
            {nm: np.asarray(out_arrs[i]).reshape(NCORES, *out_avals[i].shape)[c]
             for i, nm in enumerate(out_names)}
            for c in range(NCORES)]

    return run


def kernel(x, Wq, Wk, Wv, Wo):
    from concourse.bass_utils import run_bass_kernel_spmd

    # Force host numpy immediately: if the caller hands us jax arrays, any
    # .astype/.T on them would dispatch tiny jit programs to the neuron
    # backend, which wedges the device (known neuron-jit crash path).
    x, Wq, Wk, Wv, Wo = (np.asarray(a) for a in (x, Wq, Wk, Wv, Wo))

    nc = _get_nc()
    x16 = np.ascontiguousarray(x.astype(np.float16))
    Wq16 = Wq.astype(np.float16)
    Wk16 = Wk.astype(np.float16)
    Wv16 = Wv.astype(np.float16)
    Wo16 = Wo.astype(np.float16)

    xTs = [np.ascontiguousarray(x16[b].T) for b in range(B)]
    in_maps = []
    for c in range(NCORES):
        b, hg = divmod(c, HG)
        hsl = slice(OC * hg, OC * hg + OC)
        in_maps.append({
            "xt": xTs[b],
            "wq": np.ascontiguousarray(Wq16[hsl, :].T),
            "wk": np.ascontiguousarray(Wk16[hsl, :].T),
            "wv": np.ascontiguousarray(Wv16[hsl, :].T),
            "wo": np.ascontiguousarray(Wo16[:, hsl].T),
        })

    if "runner" in _CACHE:
        results = _CACHE["runner"](in_maps)
    else:
        # first call: compile + run through the sanctioned entry point,
        # then build the cached executable for subsequent calls
        results = run_bass_kernel_spmd(nc, in_maps, list(range(NCORES))).results
        _CACHE["runner"] = _make_cached_runner(nc)

    out = np.zeros((B, S, E), np.float32)
    for c in range(NCORES):
        b = c // HG
        out[b] += results[c]["outT"].T
    return out



# revision 2
# speedup vs baseline: 1.0945x; 1.0945x over previous
"""Trainium2 Bass kernel for nn_AutoregressiveSelfAttention.

Sharding (8 cores): batch (2-way) x head-group (4-way tensor parallel).
Core c: batch c//4, heads [4*(c%4), 4*(c%4)+4).
Per-core: fp16 matmuls throughout (QKV proj, scores, P@V, out-proj),
fp32 softmax statistics, fp16 partial output; host sums the 4 head-group
partials per batch (the row-parallel all-reduce) in fp32 and transposes.

Softmax without transposes: pass1 computes scores [sq, sk] only to get the
causal row-max M (fused mask+max on DVE); pass2 recomputes scores
transposed with the max-subtraction folded in as a rank-1 matmul term
([kT;1].T @ [qT;-M]), exps on ACT into fp16, and the ctx matmul against
[v|1] accumulates both ctx and the softmax denominator in one PSUM tile.

v2 restructure vs baseline:
- q/k projections run contraction-outer across all 8 PSUM banks so the
  first matmuls start as soon as the first xt chunk lands (DMA-paced).
- Input DMAs batched per tensor (wq, wk, xt x8, wv, wo); augmented q/k
  copies batched to one [64, S] DMA per (tensor, head) issued upfront.
- Engine rebalance: projection PSUM->SBUF copies and out-proj copies on
  ACT (was DVE), -M negate folded into an ACT copy (scale=-1), DVE keeps
  only the row-max reduces, reciprocals and normalize multiplies.
- Emission order software-pipelines across chunks: pass1(pr1) hides
  mrow(pr0) DMA latency, vproj(j+1) hides normalize latency before
  outproj(j).
- Output stored fp16 and written with one batched DMA per seq chunk.
"""
import sys
sys.path.insert(0, "/opt/trn_rl_repo")

import math
import numpy as np

B, S, E, H, D = 2, 2048, 1024, 16, 64
NCORES = 8
HG = 4                  # head-group shards
HPC = H // HG           # 4 heads per core
OC = HPC * D            # 256 per-core projection width
P = 128
NK = E // P             # 8 contraction tiles for projections
NT = S // P             # 16 seq tiles of 128
NJ = S // 512           # 4 seq chunks of 512

_CACHE = {}


def _build():
    import concourse.bacc as bacc
    import concourse.mybir as mybir
    import concourse.tile as tile
    from concourse.masks import make_identity, make_causal_mask

    dt = mybir.dt
    f32, f16 = dt.float32, dt.float16
    AX = mybir.AxisListType.X
    ALU = mybir.AluOpType
    COPY = mybir.ActivationFunctionType.Copy
    EXP = mybir.ActivationFunctionType.Exp

    nc = bacc.Bacc(None, target_bir_lowering=False, debug=False)
    with tile.TileContext(nc) as tc:
        with tc.tile_pool(name="dram", bufs=1, space="DRAM") as dram, \
             tc.tile_pool(name="persist", bufs=1) as pers, \
             tc.tile_pool(name="stream", bufs=4) as strm, \
             tc.tile_pool(name="tmp", bufs=4) as tmp, \
             tc.tile_pool(name="ps", bufs=1, space="PSUM") as ps:

            # ---- DRAM I/O ----
            xt = dram.tile([E, S], f16, kind="ExternalInput", name="xt", uniquify=False)
            wq = dram.tile([E, OC], f16, kind="ExternalInput", name="wq", uniquify=False)
            wk = dram.tile([E, OC], f16, kind="ExternalInput", name="wk", uniquify=False)
            wv = dram.tile([E, OC], f16, kind="ExternalInput", name="wv", uniquify=False)
            wo = dram.tile([OC, E], f16, kind="ExternalInput", name="wo", uniquify=False)
            outT = dram.tile([E, S], f16, kind="ExternalOutput", name="outT", uniquify=False)

            # ---- persistent SBUF ----
            xt_sb = pers.tile([P, NK, S], f16)
            wq_sb = pers.tile([P, NK, OC], f16)
            wk_sb = pers.tile([P, NK, OC], f16)
            wv_sb = pers.tile([P, NK, OC], f16)
            wo_sb = pers.tile([P, 2, E], f16)
            qp_sb = pers.tile([P, 2, S], f16)       # qT, head-pair stacked
            kp_sb = pers.tile([P, 2, S], f16)       # kT, head-pair stacked
            qaug = pers.tile([65, HPC, S], f16)     # [qT_h ; -M_h] per head
            kaug = pers.tile([65, HPC, S], f16)     # [kT_h ; ones] per head
            vv0 = pers.tile([P, NT, 2, 65], f16)    # heads 0,2: [v(0:64) | ones]
            vv1 = pers.tile([P, NT, 2, P], f16)     # heads 1,3: [ones|0*63|v(64:128)]
            ctxn = pers.tile([P, 2, S], f16)        # normalized ctx, pair stacked
            m2 = pers.tile([P, 2, 32], f32)         # rowmax cols per pair (hh*16+t)
            ident = pers.tile([P, P], f32)
            ident16 = pers.tile([P, P], f16)
            cmask16 = pers.tile([P, P], f16)        # 0 / -30000 causal block

            # ---- input DMAs (batched, consumer order) ----
            xt_v = xt[:].rearrange("(k p) s -> p k s", p=P)
            nc.sync.dma_start(out=wq_sb[:, :, :],
                              in_=wq[:].rearrange("(k p) o -> p k o", p=P))
            nc.sync.dma_start(out=wk_sb[:, :, :],
                              in_=wk[:].rearrange("(k p) o -> p k o", p=P))
            for k in range(NK):
                nc.sync.dma_start(out=xt_sb[:, k, :], in_=xt_v[:, k, :])
            nc.sync.dma_start(out=wv_sb[:, :, :],
                              in_=wv[:].rearrange("(k p) o -> p k o", p=P))
            nc.sync.dma_start(out=wo_sb[:, :, :],
                              in_=wo[:].rearrange("(k p) e -> p k e", p=P))

            # ---- constants ----
            make_identity(nc, ident[:, :])
            make_identity(nc, ident16[:, :])
            make_causal_mask(nc, cmask16[:, :], mask_val=-30000.0)
            nc.gpsimd.memset(kaug[64:65, :, :], 1.0)
            nc.gpsimd.memset(vv0[:, :, :, 64:65], 1.0)
            nc.gpsimd.memset(vv1[:, :, :, 0:1], 1.0)
            nc.gpsimd.memset(vv1[:, :, :, 1:64], 0.0)

            # ---- q/k projections: contraction-outer over all 8 PSUM banks ----
            TAGS8 = ["s1", "s1", "s2", "s2", "ctx", "ctx", "proj", "proj"]
            for dst, w_sb in ((qp_sb, wq_sb), (kp_sb, wk_sb)):
                pp = []
                for i in range(8):
                    t_ = ps.tile([P, 512], f32, tag=TAGS8[i], bufs=2,
                                 name=f"pp{i}")
                    pp.append(t_)
                for k in range(NK):
                    for i in range(8):
                        ot, j = divmod(i, NJ)
                        nc.tensor.matmul(
                            pp[i][:, :],
                            w_sb[:, k, 128 * ot:128 * ot + 128],
                            xt_sb[:, k, 512 * j:512 * j + 512],
                            start=(k == 0), stop=(k == NK - 1))
                for i in range(8):
                    ot, j = divmod(i, NJ)
                    sl = slice(512 * j, 512 * j + 512)
                    if i % 2 == 0:
                        nc.vector.tensor_copy(dst[:, ot, sl], pp[i][:, :])
                    else:
                        nc.scalar.copy(dst[:, ot, sl], pp[i][:, :])

            # ---- augmented qT/kT rows 0:64: one big DMA per (tensor, head) ----
            for h in range(HPC):
                pr, hh = divmod(h, 2)
                nc.sync.dma_start(out=qaug[0:64, h, :],
                                  in_=qp_sb[64 * hh:64 * hh + 64, pr, :])
                nc.sync.dma_start(out=kaug[0:64, h, :],
                                  in_=kp_sb[64 * hh:64 * hh + 64, pr, :])

            def emit_vproj(st):
                pv = ps.tile([P, OC], f32, tag="proj", bufs=2)
                for k in range(NK):
                    nc.tensor.matmul(
                        pv[:, :], xt_sb[:, k, P * st:P * st + P], wv_sb[:, k, :],
                        start=(k == 0), stop=(k == NK - 1))
                pv4 = pv[:, :].rearrange("p (g x d) -> p g x d", g=2, x=2)
                nc.scalar.copy(vv0[:, st, :, 0:64], pv4[:, :, 0, :])
                nc.scalar.copy(vv1[:, st, :, 64:P], pv4[:, :, 1, :])

            def emit_pass1(pr, t):
                # scores [sq, sk] for one sq-tile, 2-head tile-packed; fused
                # (+causal mask) -> rowmax into m2 columns.
                ncols = (t + 1) * P
                nch = (ncols + 511) // 512
                if nch > 1:
                    m4a = tmp.tile([P, 4], f32, tag="m4a", bufs=2)
                    m4b = tmp.tile([P, 4], f32, tag="m4b", bufs=2)
                for c in range(nch):
                    n = min(512, ncols - 512 * c)
                    sa = ps.tile([P, 512], f32, tag="s1", bufs=2)
                    sb_ = ps.tile([P, 512], f32, tag="s1", bufs=2)
                    last = c == nch - 1
                    nc.tensor.matmul(
                        sa[:, :n], qp_sb[0:64, pr, P * t:P * t + P],
                        kp_sb[0:64, pr, 512 * c:512 * c + n],
                        start=True, stop=not last, tile_position=(0, 0))
                    nc.tensor.matmul(
                        sb_[:, :n], qp_sb[64:P, pr, P * t:P * t + P],
                        kp_sb[64:P, pr, 512 * c:512 * c + n],
                        start=True, stop=not last, tile_position=(64, 0))
                    if last:
                        doff = n - P
                        nc.tensor.matmul(sa[:, doff:doff + P], ident16[:, :],
                                         cmask16[:, :], start=False, stop=True)
                        nc.tensor.matmul(sb_[:, doff:doff + P], ident16[:, :],
                                         cmask16[:, :], start=False, stop=True)
                    if nch == 1:
                        nc.vector.reduce_max(m2[:, pr, t:t + 1], sa[:, :n],
                                             axis=AX)
                        nc.vector.reduce_max(m2[:, pr, 16 + t:16 + t + 1],
                                             sb_[:, :n], axis=AX)
                    else:
                        nc.vector.reduce_max(m4a[:, c:c + 1], sa[:, :n], axis=AX)
                        nc.vector.reduce_max(m4b[:, c:c + 1], sb_[:, :n], axis=AX)
                if nch > 1:
                    nc.vector.reduce_max(m2[:, pr, t:t + 1], m4a[:, 0:nch],
                                         axis=AX)
                    nc.vector.reduce_max(m2[:, pr, 16 + t:16 + t + 1],
                                         m4b[:, 0:nch], axis=AX)

            def emit_mrow(pr, j):
                # -M for chunk j's four sq-tiles -> row 64 of qaug, per head.
                for hh in range(2):
                    mt_ps = ps.tile([4, P], f32, tag="s1", bufs=2)
                    nc.tensor.transpose(
                        mt_ps[:, :], m2[:, pr, 16 * hh + 4 * j:16 * hh + 4 * j + 4],
                        ident[:, :])
                    mt_t = tmp.tile([4, P], f16, tag="mt", bufs=2)
                    nc.scalar.activation(mt_t[:, :], mt_ps[:, :], COPY,
                                         scale=-1.0)
                    nc.sync.dma_start(
                        out=qaug[64:65, 2 * pr + hh,
                                 512 * j:512 * j + 512].rearrange(
                                     "q (t p) -> q t p", t=4),
                        in_=mt_t[:, :])

            def emit_pass2(h, j):
                # scoresT with folded -M, exp, causal zeroing, ctx+rowsum
                # accumulation, and normalization into ctxn.
                pr, hh = divmod(h, 2)
                ctxp = ps.tile([P, 512], f32, tag="ctx", bufs=2)
                nt_here = 4 * j + 4
                for t in range(nt_here):
                    if t < 4 * j:
                        qoff, n = 512 * j, 512
                    else:
                        qoff = P * t
                        n = 512 * j + 512 - P * t
                    s2p = ps.tile([P, 512], f32, tag="s2", bufs=2)
                    nc.tensor.matmul(
                        s2p[:, :n], kaug[:, h, P * t:P * t + P],
                        qaug[:, h, qoff:qoff + n], start=True, stop=True)
                    pt = strm.tile([P, 512], f16, tag="pt", bufs=6)
                    nc.scalar.activation(pt[:, :n], s2p[:, :n], EXP, scale=8.0)
                    if t >= 4 * j:
                        # zero strictly-upper block at the diagonal
                        nc.gpsimd.affine_select(
                            out=pt[:, 0:P], in_=pt[:, 0:P],
                            compare_op=ALU.is_ge, fill=0.0, base=0,
                            pattern=[[1, P]], channel_multiplier=-1)
                    lhsT = vv0[:, t, pr, :] if hh == 0 else vv1[:, t, pr, :]
                    nc.tensor.matmul(
                        ctxp[0:(65 if hh == 0 else P),
                             qoff - 512 * j:qoff - 512 * j + n],
                        lhsT, pt[:, :n],
                        start=(t == 0), stop=(t == nt_here - 1))
                # normalize: ctx / rowsum
                rsrow = 64 if hh == 0 else 0
                rr = tmp.tile([65, 512], f32, tag="rr", bufs=2)
                nc.vector.reciprocal(rr[rsrow:rsrow + 1, :],
                                     ctxp[rsrow:rsrow + 1, :])
                rb = tmp.tile([P, 512], f32, tag="rb", bufs=2)
                nc.sync.dma_start(
                    out=rb[64 * hh:64 * hh + 64, :],
                    in_=rr[rsrow:rsrow + 1, :].unsqueeze(1).broadcast_to(
                        (1, 64, 512)))
                nc.vector.tensor_mul(
                    ctxn[64 * hh:64 * hh + 64, pr, 512 * j:512 * j + 512],
                    ctxp[64 * hh:64 * hh + 64, :],
                    rb[64 * hh:64 * hh + 64, :])

            outT_v = outT[:].rearrange("(o p) s -> p o s", p=P)

            def emit_outproj(j):
                ob = strm.tile([P, NK, 512], f16, tag="ob", bufs=2)
                for oo in range(E // P):
                    po = ps.tile([P, 512], f32, tag="proj", bufs=2)
                    for kt in range(2):
                        nc.tensor.matmul(
                            po[:, :], wo_sb[:, kt, P * oo:P * oo + P],
                            ctxn[:, kt, 512 * j:512 * j + 512],
                            start=(kt == 0), stop=(kt == 1))
                    nc.scalar.copy(ob[:, oo, :], po[:, :])
                nc.sync.dma_start(out=outT_v[:, :, 512 * j:512 * j + 512],
                                  in_=ob[:, :, :])

            # ---- software-pipelined attention ----
            for st in range(4):
                emit_vproj(st)
            for j in range(NJ):
                for pr in range(2):
                    for t in range(4 * j, 4 * j + 4):
                        emit_pass1(pr, t)
                    emit_mrow(pr, j)
                for h in range(HPC):
                    emit_pass2(h, j)
                if j < NJ - 1:
                    for st in range(4 * (j + 1), 4 * (j + 1) + 4):
                        emit_vproj(st)
                emit_outproj(j)

    nc.compile()
    return nc


def _get_nc():
    if "nc" not in _CACHE:
        _CACHE["nc"] = _build()
    return _CACHE["nc"]


def _make_cached_runner(nc):
    """Trace/compile the 8-core PJRT executable once; reuse on later calls.

    Mirrors concourse.bass2jax.run_bass_via_pjrt's multi-core branch, but
    keeps the jitted shard_map so repeat kernel() calls skip re-trace and
    re-lowering (the NEFF itself is already cached by neuronx_cc_hook).
    """
    import jax
    import jax.numpy as jnp
    from jax.sharding import Mesh, PartitionSpec
    from jax.experimental.shard_map import shard_map
    from concourse import bass2jax, mybir

    bass2jax.install_neuronx_cc_hook()
    partition_name = nc.partition_id_tensor.name if nc.partition_id_tensor else None
    in_names, out_names, out_avals = [], [], []
    for alloc in nc.m.functions[0].allocations:
        if not isinstance(alloc, mybir.MemoryLocationSet):
            continue
        name = alloc.memorylocations[0].name
        if alloc.kind == "ExternalInput":
            if name != partition_name:
                in_names.append(name)
        elif alloc.kind == "ExternalOutput":
            out_names.append(name)
            out_avals.append(jax.core.ShapedArray(
                tuple(alloc.tensor_shape), mybir.dt.np(alloc.dtype)))
    n_params = len(in_names)
    n_outs = len(out_avals)
    all_names = list(in_names) + list(out_names)
    if partition_name is not None:
        all_names.append(partition_name)

    def _body(*args):
        operands = list(args)
        if partition_name is not None:
            operands.append(bass2jax.partition_id_tensor())
        outs = bass2jax._bass_exec_p.bind(
            *operands,
            out_avals=tuple(out_avals),
            in_names=tuple(all_names),
            out_names=tuple(out_names),
            lowering_input_output_aliases=(),
            sim_require_finite=True,
            sim_require_nnan=True,
            nc=nc,
        )
        return tuple(outs)

    devices = jax.devices()[:NCORES]
    mesh = Mesh(np.asarray(devices), ("core",))
    in_specs = (PartitionSpec("core"),) * (n_params + n_outs)
    out_specs = (PartitionSpec("core"),) * n_outs
    sharded = jax.jit(
        shard_map(_body, mesh=mesh, in_specs=in_specs, out_specs=out_specs,
                  check_rep=False),
        donate_argnums=tuple(range(n_params, n_params + n_outs)),
        keep_unused=True)

    def run(in_maps):
        concat_in = [
            np.concatenate([np.asarray(in_maps[c][nm]) for c in range(NCORES)],
                           axis=0)
            for nm in in_names]
        concat_zeros = [
            np.zeros((NCORES * a.shape[0], *a.shape[1:]), a.dtype)
            for a in out_avals]
        out_arrs = sharded(*concat_in, *concat_zeros)
        return [
            {nm: np.asarray(out_arrs[i]).reshape(NCORES, *out_avals[i].shape)[c]
             for i, nm in enumerate(out_names)}
            for c in range(NCORES)]

    return run


def kernel(x, Wq, Wk, Wv, Wo):
    from concourse.bass_utils import run_bass_kernel_spmd

    # Force host numpy immediately: if the caller hands us jax arrays, any
    # .astype/.T on them would dispatch tiny jit programs to the neuron
    # backend, which wedges the device (known neuron-jit crash path).
    x, Wq, Wk, Wv, Wo = (np.asarray(a) for a in (x, Wq, Wk, Wv, Wo))

    nc = _get_nc()
    x16 = np.ascontiguousarray(x.astype(np.float16))
    Wq16 = Wq.astype(np.float16)
    Wk16 = Wk.astype(np.float16)
    Wv16 = Wv.astype(np.float16)
    Wo16 = Wo.astype(np.float16)

    xTs = [np.ascontiguousarray(x16[b].T) for b in range(B)]
    in_maps = []
    for c in range(NCORES):
        b, hg = divmod(c, HG)
        hsl = slice(OC * hg, OC * hg + OC)
        in_maps.append({
            "xt": xTs[b],
            "wq": np.ascontiguousarray(Wq16[hsl, :].T),
            "wk": np.ascontiguousarray(Wk16[hsl, :].T),
            "wv": np.ascontiguousarray(Wv16[hsl, :].T),
            "wo": np.ascontiguousarray(Wo16[:, hsl].T),
        })

    if "runner" in _CACHE:
        results = _CACHE["runner"](in_maps)
    else:
        # first call: compile + run through the sanctioned entry point,
        # then build the cached executable for subsequent calls
        results = run_bass_kernel_spmd(nc, in_maps, list(range(NCORES))).results
        _CACHE["runner"] = _make_cached_runner(nc)

    out = np.zeros((B, S, E), np.float32)
    for c in range(NCORES):
        b = c // HG
        out[b] += results[c]["outT"].T.astype(np.float32)
    return out


# revision 4
# speedup vs baseline: 1.1239x; 1.0268x over previous
"""Trainium2 Bass kernel for nn_AutoregressiveSelfAttention.

Sharding (8 cores): batch (2-way) x head-group (4-way tensor parallel).
Core c: batch c//4, heads [4*(c%4), 4*(c%4)+4).
Per-core: fp16 matmuls throughout (QKV proj, scores, P@V, out-proj),
fp32 softmax statistics, fp16 partial output; host sums the 4 head-group
partials per batch (the row-parallel all-reduce) in fp32 and transposes.

Softmax without transposes: pass1 computes scores [sq, sk] only to get the
causal row-max M (fused mask+max on DVE); pass2 recomputes scores
transposed with the max-subtraction folded in as a rank-1 matmul term
([kT;1].T @ [qT;-M]), exps on ACT into fp16, and the ctx matmul against
[v|1] accumulates both ctx and the softmax denominator in one PSUM tile.

v2 restructure vs baseline:
- q/k projections run contraction-outer across all 8 PSUM banks so the
  first matmuls start as soon as the first xt chunk lands (DMA-paced).
- Input DMAs batched per tensor (wq, wk, xt x8, wv, wo); augmented q/k
  copies batched to one [64, S] DMA per (tensor, head) issued upfront.
- Engine rebalance: projection PSUM->SBUF copies and out-proj copies on
  ACT (was DVE), -M negate folded into an ACT copy (scale=-1), DVE keeps
  only the row-max reduces, reciprocals and normalize multiplies.
- Emission order software-pipelines across chunks: pass1(pr1) hides
  mrow(pr0) DMA latency, vproj(j+1) hides normalize latency before
  outproj(j).
- Output stored fp16 and written with one batched DMA per seq chunk.
"""
import sys
sys.path.insert(0, "/opt/trn_rl_repo")

import math
import numpy as np

B, S, E, H, D = 2, 2048, 1024, 16, 64
NCORES = 8
HG = 4                  # head-group shards
HPC = H // HG           # 4 heads per core
OC = HPC * D            # 256 per-core projection width
P = 128
NK = E // P             # 8 contraction tiles for projections
NT = S // P             # 16 seq tiles of 128
NJ = S // 512           # 4 seq chunks of 512

_CACHE = {}


def _build():
    import concourse.bacc as bacc
    import concourse.mybir as mybir
    import concourse.tile as tile
    from concourse.masks import make_identity, make_causal_mask

    dt = mybir.dt
    f32, f16 = dt.float32, dt.float16
    AX = mybir.AxisListType.X
    ALU = mybir.AluOpType
    COPY = mybir.ActivationFunctionType.Copy
    EXP = mybir.ActivationFunctionType.Exp

    nc = bacc.Bacc(None, target_bir_lowering=False, debug=False)
    with tile.TileContext(nc) as tc:
        with tc.tile_pool(name="dram", bufs=1, space="DRAM") as dram, \
             tc.tile_pool(name="persist", bufs=1) as pers, \
             tc.tile_pool(name="stream", bufs=4) as strm, \
             tc.tile_pool(name="tmp", bufs=4) as tmp, \
             tc.tile_pool(name="ps", bufs=1, space="PSUM") as ps:

            # ---- DRAM I/O ----
            xt = dram.tile([E, S], f16, kind="ExternalInput", name="xt", uniquify=False)
            wq = dram.tile([E, OC], f16, kind="ExternalInput", name="wq", uniquify=False)
            wk = dram.tile([E, OC], f16, kind="ExternalInput", name="wk", uniquify=False)
            wv = dram.tile([E, OC], f16, kind="ExternalInput", name="wv", uniquify=False)
            wo = dram.tile([OC, E], f16, kind="ExternalInput", name="wo", uniquify=False)
            outT = dram.tile([E, S], f16, kind="ExternalOutput", name="outT", uniquify=False)

            # ---- persistent SBUF ----
            xt_sb = pers.tile([P, NK, S], f16)
            wq_sb = pers.tile([P, NK, OC], f16)
            wk_sb = pers.tile([P, NK, OC], f16)
            wv_sb = pers.tile([P, NK, OC], f16)
            wo_sb = pers.tile([P, 2, E], f16)
            qp_sb = pers.tile([P, 2, S], f16)       # qT, head-pair stacked
            kp_sb = pers.tile([P, 2, S], f16)       # kT, head-pair stacked
            qaug = pers.tile([65, HPC, S], f16)     # [qT_h ; -M_h] per head
            kaug = pers.tile([65, HPC, S], f16)     # [kT_h ; ones] per head
            vv0 = pers.tile([P, NT, 2, 65], f16)    # heads 0,2: [v(0:64) | ones]
            vv1 = pers.tile([P, NT, 2, P], f16)     # heads 1,3: [ones|0*63|v(64:128)]
            ctxn = pers.tile([P, 2, S], f16)        # normalized ctx, pair stacked
            m2 = pers.tile([P, 2, 32], f32)         # rowmax cols per pair (hh*16+t)
            ident = pers.tile([P, P], f32)
            ident16 = pers.tile([P, P], f16)
            cmask16 = pers.tile([P, P], f16)        # 0 / -30000 causal block

            # ---- input DMAs (batched, consumer order) ----
            xt_v = xt[:].rearrange("(k p) s -> p k s", p=P)
            nc.sync.dma_start(out=wq_sb[:, :, :],
                              in_=wq[:].rearrange("(k p) o -> p k o", p=P))
            for k in range(NK):
                nc.sync.dma_start(out=xt_sb[:, k, :], in_=xt_v[:, k, :])
            nc.sync.dma_start(out=wk_sb[:, :, :],
                              in_=wk[:].rearrange("(k p) o -> p k o", p=P))
            nc.sync.dma_start(out=wv_sb[:, :, :],
                              in_=wv[:].rearrange("(k p) o -> p k o", p=P))
            nc.sync.dma_start(out=wo_sb[:, :, :],
                              in_=wo[:].rearrange("(k p) e -> p k e", p=P))

            # ---- constants ----
            make_identity(nc, ident[:, :])
            make_identity(nc, ident16[:, :])
            make_causal_mask(nc, cmask16[:, :], mask_val=-30000.0)
            nc.gpsimd.memset(kaug[64:65, :, :], 1.0)
            nc.gpsimd.memset(vv0[:, :, :, 64:65], 1.0)
            nc.gpsimd.memset(vv1[:, :, :, 0:1], 1.0)
            nc.gpsimd.memset(vv1[:, :, :, 1:64], 0.0)

            # ---- q/k projections: contraction-outer over all 8 PSUM banks ----
            TAGS8 = ["s1", "s1", "s2", "s2", "ctx", "ctx", "proj", "proj"]
            for dst, w_sb in ((qp_sb, wq_sb), (kp_sb, wk_sb)):
                pp = []
                for i in range(8):
                    t_ = ps.tile([P, 512], f32, tag=TAGS8[i], bufs=2,
                                 name=f"pp{i}")
                    pp.append(t_)
                for k in range(NK):
                    for i in range(8):
                        ot, j = divmod(i, NJ)
                        nc.tensor.matmul(
                            pp[i][:, :],
                            w_sb[:, k, 128 * ot:128 * ot + 128],
                            xt_sb[:, k, 512 * j:512 * j + 512],
                            start=(k == 0), stop=(k == NK - 1))
                for i in range(8):
                    ot, j = divmod(i, NJ)
                    sl = slice(512 * j, 512 * j + 512)
                    if i % 2 == 0:
                        nc.vector.tensor_copy(dst[:, ot, sl], pp[i][:, :])
                    else:
                        nc.scalar.copy(dst[:, ot, sl], pp[i][:, :])

            # ---- augmented qT/kT rows 0:64: one big DMA per (tensor, head) ----
            for h in range(HPC):
                pr, hh = divmod(h, 2)
                nc.sync.dma_start(out=qaug[0:64, h, :],
                                  in_=qp_sb[64 * hh:64 * hh + 64, pr, :])
                nc.sync.dma_start(out=kaug[0:64, h, :],
                                  in_=kp_sb[64 * hh:64 * hh + 64, pr, :])

            # Emission below is organized as lists of closures ("streams")
            # that are proportionally interleaved: pass2(j) (ACT-paced) is
            # merged instruction-by-instruction with pass1(j+1) (DVE-paced),
            # vproj(j+1) and outproj(j-1) (PE-pure / mixed) so that every
            # engine has ready work throughout the chunk.

            def vproj_closures(st):
                st_ = {}

                def c1():
                    pv = ps.tile([P, OC], f32, tag="proj", bufs=2, name="pv")
                    st_["pv"] = pv
                    for k in range(4):
                        nc.tensor.matmul(
                            pv[:, :], xt_sb[:, k, P * st:P * st + P],
                            wv_sb[:, k, :], start=(k == 0), stop=False)

                def c2():
                    pv = st_["pv"]
                    for k in range(4, NK):
                        nc.tensor.matmul(
                            pv[:, :], xt_sb[:, k, P * st:P * st + P],
                            wv_sb[:, k, :], start=False, stop=(k == NK - 1))

                def c3():
                    pv4 = st_["pv"][:, :].rearrange("p (g x d) -> p g x d",
                                                    g=2, x=2)
                    nc.scalar.copy(vv0[:, st, :, 0:64], pv4[:, :, 0, :])
                    nc.scalar.copy(vv1[:, st, :, 64:P], pv4[:, :, 1, :])

                return [c1, c2, c3]

            def pass1_closures(pr, t):
                # scores [sq, sk] for one sq-tile, 2-head tile-packed; fused
                # (+causal mask) -> rowmax into m2 columns.
                ncols = (t + 1) * P
                nch = (ncols + 511) // 512
                st_ = {}

                def mk(c):
                    def cl():
                        n = min(512, ncols - 512 * c)
                        sa = ps.tile([P, 512], f32, tag="s1", bufs=2, name="sa")
                        sb_ = ps.tile([P, 512], f32, tag="s1", bufs=2,
                                      name="sb_")
                        last = c == nch - 1
                        if c == 0 and nch > 1:
                            st_["m4a"] = tmp.tile([P, 4], f32, tag="m4a",
                                                  bufs=2, name="m4a")
                            st_["m4b"] = tmp.tile([P, 4], f32, tag="m4b",
                                                  bufs=2, name="m4b")
                        nc.tensor.matmul(
                            sa[:, :n], qp_sb[0:64, pr, P * t:P * t + P],
                            kp_sb[0:64, pr, 512 * c:512 * c + n],
                            start=True, stop=not last, tile_position=(0, 0))
                        nc.tensor.matmul(
                            sb_[:, :n], qp_sb[64:P, pr, P * t:P * t + P],
                            kp_sb[64:P, pr, 512 * c:512 * c + n],
                            start=True, stop=not last, tile_position=(64, 0))
                        if last:
                            doff = n - P
                            nc.tensor.matmul(sa[:, doff:doff + P],
                                             ident16[:, :], cmask16[:, :],
                                             start=False, stop=True)
                            nc.tensor.matmul(sb_[:, doff:doff + P],
                                             ident16[:, :], cmask16[:, :],
                                             start=False, stop=True)
                        if nch == 1:
                            nc.vector.reduce_max(m2[:, pr, t:t + 1], sa[:, :n],
                                                 axis=AX)
                            nc.vector.reduce_max(m2[:, pr, 16 + t:16 + t + 1],
                                                 sb_[:, :n], axis=AX)
                        else:
                            m4a, m4b = st_["m4a"], st_["m4b"]
                            nc.vector.reduce_max(m4a[:, c:c + 1], sa[:, :n],
                                                 axis=AX)
                            nc.vector.reduce_max(m4b[:, c:c + 1], sb_[:, :n],
                                                 axis=AX)
                            if last:
                                nc.vector.reduce_max(m2[:, pr, t:t + 1],
                                                     m4a[:, 0:nch], axis=AX)
                                nc.vector.reduce_max(
                                    m2[:, pr, 16 + t:16 + t + 1],
                                    m4b[:, 0:nch], axis=AX)
                    return cl

                return [mk(c) for c in range(nch)]

            def mrow_closure(pr, j):
                # -M for chunk j's four sq-tiles -> row 64 of qaug, per head.
                def cl():
                    for hh in range(2):
                        mt_ps = ps.tile([4, P], f32, tag="s1", bufs=2,
                                        name="mt_ps")
                        nc.tensor.transpose(
                            mt_ps[:, :],
                            m2[:, pr, 16 * hh + 4 * j:16 * hh + 4 * j + 4],
                            ident[:, :])
                        mt_t = tmp.tile([4, P], f16, tag="mt", bufs=2,
                                        name="mt_t")
                        nc.scalar.activation(mt_t[:, :], mt_ps[:, :], COPY,
                                             scale=-1.0)
                        nc.sync.dma_start(
                            out=qaug[64:65, 2 * pr + hh,
                                     512 * j:512 * j + 512].rearrange(
                                         "q (t p) -> q t p", t=4),
                            in_=mt_t[:, :])
                return [cl]

            def pass2_closures(h, j):
                # scoresT with folded -M, exp, causal zeroing, ctx+rowsum
                # accumulation, and normalization into ctxn.
                pr, hh = divmod(h, 2)
                nt_here = 4 * j + 4
                st_ = {}

                def mk(t):
                    def cl():
                        if t == 0:
                            st_["ctxp"] = ps.tile([P, 512], f32, tag="ctx",
                                                  bufs=2, name="ctxp")
                        ctxp = st_["ctxp"]
                        if t < 4 * j:
                            qoff, n = 512 * j, 512
                        else:
                            qoff = P * t
                            n = 512 * j + 512 - P * t
                        s2p = ps.tile([P, 512], f32, tag="s2", bufs=2,
                                      name="s2p")
                        nc.tensor.matmul(
                            s2p[:, :n], kaug[:, h, P * t:P * t + P],
                            qaug[:, h, qoff:qoff + n], start=True, stop=True)
                        pt = strm.tile([P, 512], f16, tag="pt", bufs=6,
                                       name="pt")
                        nc.scalar.activation(pt[:, :n], s2p[:, :n], EXP,
                                             scale=8.0)
                        if t >= 4 * j:
                            # zero strictly-upper block at the diagonal
                            nc.gpsimd.affine_select(
                                out=pt[:, 0:P], in_=pt[:, 0:P],
                                compare_op=ALU.is_ge, fill=0.0, base=0,
                                pattern=[[1, P]], channel_multiplier=-1)
                        lhsT = vv0[:, t, pr, :] if hh == 0 else vv1[:, t, pr, :]
                        nc.tensor.matmul(
                            ctxp[0:(65 if hh == 0 else P),
                                 qoff - 512 * j:qoff - 512 * j + n],
                            lhsT, pt[:, :n],
                            start=(t == 0), stop=(t == nt_here - 1))
                    return cl

                def norm():
                    # normalize: ctx / rowsum
                    ctxp = st_["ctxp"]
                    rsrow = 64 if hh == 0 else 0
                    rr = tmp.tile([65, 512], f32, tag="rr", bufs=2, name="rr")
                    nc.vector.reciprocal(rr[rsrow:rsrow + 1, :],
                                         ctxp[rsrow:rsrow + 1, :])
                    rb = tmp.tile([P, 512], f32, tag="rb", bufs=2, name="rb")
                    nc.sync.dma_start(
                        out=rb[64 * hh:64 * hh + 64, :],
                        in_=rr[rsrow:rsrow + 1, :].unsqueeze(1).broadcast_to(
                            (1, 64, 512)))
                    nc.vector.tensor_mul(
                        ctxn[64 * hh:64 * hh + 64, pr, 512 * j:512 * j + 512],
                        ctxp[64 * hh:64 * hh + 64, :],
                        rb[64 * hh:64 * hh + 64, :])

                return [mk(t) for t in range(nt_here)] + [norm]

            outT_v = outT[:].rearrange("(o p) s -> p o s", p=P)

            def outproj_closures(j):
                st_ = {}

                def mk(oo):
                    def cl():
                        if oo == 0:
                            st_["ob"] = strm.tile([P, NK, 512], f16, tag="ob",
                                                  bufs=2, name="ob")
                        ob = st_["ob"]
                        po = ps.tile([P, 512], f32, tag="proj", bufs=2,
                                     name="po")
                        for kt in range(2):
                            nc.tensor.matmul(
                                po[:, :], wo_sb[:, kt, P * oo:P * oo + P],
                                ctxn[:, kt, 512 * j:512 * j + 512],
                                start=(kt == 0), stop=(kt == 1))
                        if oo % 2 == 0:
                            nc.scalar.copy(ob[:, oo, :], po[:, :])
                        else:
                            nc.vector.tensor_copy(ob[:, oo, :], po[:, :])
                        if oo == 3:
                            nc.sync.dma_start(
                                out=outT_v[:, 0:4, 512 * j:512 * j + 512],
                                in_=ob[:, 0:4, :])
                        elif oo == 7:
                            nc.sync.dma_start(
                                out=outT_v[:, 4:NK, 512 * j:512 * j + 512],
                                in_=ob[:, 4:NK, :])
                    return cl

                return [mk(oo) for oo in range(E // P)]

            def merge_emit(a_ops, b_ops):
                bq = list(b_ops)
                if not a_ops:
                    for b in bq:
                        b()
                    return
                ratio = len(bq) / len(a_ops)
                acc = 0.0
                for a in a_ops:
                    a()
                    acc += ratio
                    while acc >= 1.0 and bq:
                        bq.pop(0)()
                        acc -= 1.0
                for b in bq:
                    b()

            # ---- software-pipelined attention ----
            lead = []
            for st in range(4):
                lead += vproj_closures(st)
            for pr in range(2):
                for t in range(4):
                    lead += pass1_closures(pr, t)
                lead += mrow_closure(pr, 0)
            for cl in lead:
                cl()
            for j in range(NJ):
                a_ops = []
                for h in range(HPC):
                    a_ops += pass2_closures(h, j)
                b_ops = []
                if j > 0:
                    b_ops += outproj_closures(j - 1)
                if j < NJ - 1:
                    for st in range(4 * (j + 1), 4 * (j + 1) + 4):
                        b_ops += vproj_closures(st)
                    for pr in range(2):
                        for t in range(4 * (j + 1), 4 * (j + 1) + 4):
                            b_ops += pass1_closures(pr, t)
                        b_ops += mrow_closure(pr, j + 1)
                merge_emit(a_ops, b_ops)
            for cl in outproj_closures(NJ - 1):
                cl()

    nc.compile()
    return nc


def _get_nc():
    if "nc" not in _CACHE:
        _CACHE["nc"] = _build()
    return _CACHE["nc"]


def _make_cached_runner(nc):
    """Trace/compile the 8-core PJRT executable once; reuse on later calls.

    Mirrors concourse.bass2jax.run_bass_via_pjrt's multi-core branch, but
    keeps the jitted shard_map so repeat kernel() calls skip re-trace and
    re-lowering (the NEFF itself is already cached by neuronx_cc_hook).
    """
    import jax
    import jax.numpy as jnp
    from jax.sharding import Mesh, PartitionSpec
    from jax.experimental.shard_map import shard_map
    from concourse import bass2jax, mybir

    bass2jax.install_neuronx_cc_hook()
    partition_name = nc.partition_id_tensor.name if nc.partition_id_tensor else None
    in_names, out_names, out_avals = [], [], []
    for alloc in nc.m.functions[0].allocations:
        if not isinstance(alloc, mybir.MemoryLocationSet):
            continue
        name = alloc.memorylocations[0].name
        if alloc.kind == "ExternalInput":
            if name != partition_name:
                in_names.append(name)
        elif alloc.kind == "ExternalOutput":
            out_names.append(name)
            out_avals.append(jax.core.ShapedArray(
                tuple(alloc.tensor_shape), mybir.dt.np(alloc.dtype)))
    n_params = len(in_names)
    n_outs = len(out_avals)
    all_names = list(in_names) + list(out_names)
    if partition_name is not None:
        all_names.append(partition_name)

    def _body(*args):
        operands = list(args)
        if partition_name is not None:
            operands.append(bass2jax.partition_id_tensor())
        outs = bass2jax._bass_exec_p.bind(
            *operands,
            out_avals=tuple(out_avals),
            in_names=tuple(all_names),
            out_names=tuple(out_names),
            lowering_input_output_aliases=(),
            sim_require_finite=True,
            sim_require_nnan=True,
            nc=nc,
        )
        return tuple(outs)

    devices = jax.devices()[:NCORES]
    mesh = Mesh(np.asarray(devices), ("core",))
    in_specs = (PartitionSpec("core"),) * (n_params + n_outs)
    out_specs = (PartitionSpec("core"),) * n_outs
    sharded = jax.jit(
        shard_map(_body, mesh=mesh, in_specs=in_specs, out_specs=out_specs,
                  check_rep=False),
        donate_argnums=tuple(range(n_params, n_params + n_outs)),
        keep_unused=True)

    def run(in_maps):
        concat_in = [
            np.concatenate([np.asarray(in_maps[c][nm]) for c in range(NCORES)],
                           axis=0)
            for nm in in_names]
        concat_zeros = [
            np.zeros((NCORES * a.shape[0], *a.shape[1:]), a.dtype)
            for a in out_avals]
        out_arrs = sharded(*concat_in, *concat_zeros)
        return [
            {nm: np.asarray(out_arrs[i]).reshape(NCORES, *out_avals[i].shape)[c]
             for i, nm in enumerate(out_names)}
            for c in range(NCORES)]

    return run


def kernel(x, Wq, Wk, Wv, Wo):
    from concourse.bass_utils import run_bass_kernel_spmd

    # Force host numpy immediately: if the caller hands us jax arrays, any
    # .astype/.T on them would dispatch tiny jit programs to the neuron
    # backend, which wedges the device (known neuron-jit crash path).
    x, Wq, Wk, Wv, Wo = (np.asarray(a) for a in (x, Wq, Wk, Wv, Wo))

    nc = _get_nc()
    x16 = np.ascontiguousarray(x.astype(np.float16))
    Wq16 = Wq.astype(np.float16)
    Wk16 = Wk.astype(np.float16)
    Wv16 = Wv.astype(np.float16)
    Wo16 = Wo.astype(np.float16)

    xTs = [np.ascontiguousarray(x16[b].T) for b in range(B)]
    in_maps = []
    for c in range(NCORES):
        b, hg = divmod(c, HG)
        hsl = slice(OC * hg, OC * hg + OC)
        in_maps.append({
            "xt": xTs[b],
            "wq": np.ascontiguousarray(Wq16[hsl, :].T),
            "wk": np.ascontiguousarray(Wk16[hsl, :].T),
            "wv": np.ascontiguousarray(Wv16[hsl, :].T),
            "wo": np.ascontiguousarray(Wo16[:, hsl].T),
        })

    if "runner" in _CACHE:
        results = _CACHE["runner"](in_maps)
    else:
        # first call: compile + run through the sanctioned entry point,
        # then build the cached executable for subsequent calls
        results = run_bass_kernel_spmd(nc, in_maps, list(range(NCORES))).results
        _CACHE["runner"] = _make_cached_runner(nc)

    out = np.zeros((B, S, E), np.float32)
    for c in range(NCORES):
        b = c // HG
        out[b] += results[c]["outT"].T.astype(np.float32)
    return out


# revision 12
# speedup vs baseline: 1.1345x; 1.0094x over previous
"""Trainium2 Bass kernel for nn_AutoregressiveSelfAttention.

Sharding (8 cores): batch (2-way) x head-group (4-way tensor parallel).
Core c: batch c//4, heads [4*(c%4), 4*(c%4)+4).
Per-core: fp16 matmuls throughout (QKV proj, scores, P@V, out-proj),
fp32 softmax statistics, fp16 partial output; host sums the 4 head-group
partials per batch (the row-parallel all-reduce) in fp32 and transposes.

Softmax without transposes: pass1 computes scores [sq, sk] only to get the
causal row-max M (fused mask+max on DVE); pass2 recomputes scores
transposed with the max-subtraction folded in as a rank-1 matmul term
([kT;1].T @ [qT;-M]), exps on ACT into fp16, and the ctx matmul against
[v|1] accumulates both ctx and the softmax denominator in one PSUM tile.

v2 restructure vs baseline:
- q/k projections run contraction-outer across all 8 PSUM banks so the
  first matmuls start as soon as the first xt chunk lands (DMA-paced).
- Input DMAs batched per tensor (wq, wk, xt x8, wv, wo); augmented q/k
  copies batched to one [64, S] DMA per (tensor, head) issued upfront.
- Engine rebalance: projection PSUM->SBUF copies and out-proj copies on
  ACT (was DVE), -M negate folded into an ACT copy (scale=-1), DVE keeps
  only the row-max reduces, reciprocals and normalize multiplies.
- Emission order software-pipelines across chunks: pass1(pr1) hides
  mrow(pr0) DMA latency, vproj(j+1) hides normalize latency before
  outproj(j).
- Output stored fp16 and written with one batched DMA per seq chunk.
"""
import sys
sys.path.insert(0, "/opt/trn_rl_repo")

import math
import numpy as np

B, S, E, H, D = 2, 2048, 1024, 16, 64
NCORES = 8
HG = 4                  # head-group shards
HPC = H // HG           # 4 heads per core
OC = HPC * D            # 256 per-core projection width
P = 128
NK = E // P             # 8 contraction tiles for projections
NT = S // P             # 16 seq tiles of 128
NJ = S // 512           # 4 seq chunks of 512

_CACHE = {}


def _build():
    import concourse.bacc as bacc
    import concourse.mybir as mybir
    import concourse.tile as tile
    from concourse.masks import make_identity, make_causal_mask

    dt = mybir.dt
    f32, f16 = dt.float32, dt.float16
    AX = mybir.AxisListType.X
    ALU = mybir.AluOpType
    COPY = mybir.ActivationFunctionType.Copy
    EXP = mybir.ActivationFunctionType.Exp

    nc = bacc.Bacc(None, target_bir_lowering=False, debug=False)
    with tile.TileContext(nc) as tc:
        with tc.tile_pool(name="dram", bufs=1, space="DRAM") as dram, \
             tc.tile_pool(name="persist", bufs=1) as pers, \
             tc.tile_pool(name="stream", bufs=4) as strm, \
             tc.tile_pool(name="tmp", bufs=4) as tmp, \
             tc.tile_pool(name="ps", bufs=1, space="PSUM") as ps:

            # ---- DRAM I/O ----
            xt = dram.tile([E, S], f16, kind="ExternalInput", name="xt", uniquify=False)
            wq = dram.tile([E, OC], f16, kind="ExternalInput", name="wq", uniquify=False)
            wk = dram.tile([E, OC], f16, kind="ExternalInput", name="wk", uniquify=False)
            wv = dram.tile([E, OC], f16, kind="ExternalInput", name="wv", uniquify=False)
            wo = dram.tile([OC, E], f16, kind="ExternalInput", name="wo", uniquify=False)
            outT = dram.tile([E, S], f16, kind="ExternalOutput", name="outT", uniquify=False)

            # ---- persistent SBUF ----
            xt_sb = pers.tile([P, NK, S], f16)
            wq_sb = pers.tile([P, NK, OC], f16)
            wk_sb = pers.tile([P, NK, OC], f16)
            wv_sb = pers.tile([P, NK, OC], f16)
            wo_sb = pers.tile([P, 2, E], f16)
            qp_sb = pers.tile([P, 2, S], f16)       # qT, head-pair stacked
            kp_sb = pers.tile([P, 2, S], f16)       # kT, head-pair stacked
            qaug = pers.tile([65, HPC, S], f16)     # [qT_h ; -M_h] per head
            kaug = pers.tile([65, HPC, S], f16)     # [kT_h ; ones] per head
            vv0 = pers.tile([P, NT, 2, 65], f16)    # heads 0,2: [v(0:64) | ones]
            vv1 = pers.tile([P, NT, 2, P], f16)     # heads 1,3: [ones|0*63|v(64:128)]
            ctxn = pers.tile([P, 2, S], f16)        # normalized ctx, pair stacked
            m2 = pers.tile([P, 2, 32], f32)         # rowmax cols per pair (hh*16+t)
            ident = pers.tile([P, P], f32)
            ident16 = pers.tile([P, P], f16)
            cmask16 = pers.tile([P, P], f16)        # 0 / -30000 above diag
            cml16 = pers.tile([P, P], f16)          # 0 / -30000 below diag

            # ---- input DMAs (batched, consumer order) ----
            xt_v = xt[:].rearrange("(k p) s -> p k s", p=P)
            nc.sync.dma_start(out=wq_sb[:, :, :],
                              in_=wq[:].rearrange("(k p) o -> p k o", p=P))
            for k in range(NK):
                nc.sync.dma_start(out=xt_sb[:, k, :], in_=xt_v[:, k, :])
            nc.sync.dma_start(out=wk_sb[:, :, :],
                              in_=wk[:].rearrange("(k p) o -> p k o", p=P))
            nc.sync.dma_start(out=wv_sb[:, :, :],
                              in_=wv[:].rearrange("(k p) o -> p k o", p=P))
            nc.sync.dma_start(out=wo_sb[:, :, :],
                              in_=wo[:].rearrange("(k p) e -> p k e", p=P))

            # ---- constants ----
            make_identity(nc, ident[:, :])
            make_identity(nc, ident16[:, :])
            make_causal_mask(nc, cmask16[:, :], mask_val=-30000.0)
            # cml16: -30000 strictly below the diagonal (masks k > q in the
            # transposed scores), built by affine-filling a zero tile.
            nc.gpsimd.memset(cml16[:, :], 0.0)
            nc.gpsimd.affine_select(
                out=cml16[:, :], in_=cml16[:, :],
                compare_op=ALU.is_ge, fill=-30000.0, base=0,
                pattern=[[1, P]], channel_multiplier=-1)
            nc.gpsimd.memset(kaug[64:65, :, :], 1.0)
            nc.gpsimd.memset(vv0[:, :, :, 64:65], 1.0)
            nc.gpsimd.memset(vv1[:, :, :, 0:1], 1.0)
            nc.gpsimd.memset(vv1[:, :, :, 1:64], 0.0)

            # ---- q projection: contraction-outer over all 8 PSUM banks so
            # matmuls pace with the xt chunk DMAs ----
            TAGS8 = ["s1", "s1", "s2", "s2", "ctx", "ctx", "proj", "proj"]
            pp = []
            for i in range(8):
                t_ = ps.tile([P, 512], f32, tag=TAGS8[i], bufs=2,
                             name=f"pp{i}")
                pp.append(t_)
            for k in range(NK):
                for i in range(8):
                    ot, j = divmod(i, NJ)
                    nc.tensor.matmul(
                        pp[i][:, :],
                        wq_sb[:, k, 128 * ot:128 * ot + 128],
                        xt_sb[:, k, 512 * j:512 * j + 512],
                        start=(k == 0), stop=(k == NK - 1))
            for i in range(8):
                ot, j = divmod(i, NJ)
                sl = slice(512 * j, 512 * j + 512)
                if i % 2 == 0:
                    nc.vector.tensor_copy(qp_sb[:, ot, sl], pp[i][:, :])
                else:
                    nc.scalar.copy(qp_sb[:, ot, sl], pp[i][:, :])

            def kproj_wave_closures(ot, tags):
                # one head-pair of the k projection: 4 tiles, k-outer
                st_ = {}

                def alloc():
                    st_["kp"] = [ps.tile([P, 512], f32, tag=tags[i], bufs=2,
                                         name=f"kpp{i}") for i in range(4)]

                def mmk(k):
                    def cl():
                        if k == 0:
                            alloc()
                        for jj in range(NJ):
                            nc.tensor.matmul(
                                st_["kp"][jj][:, :],
                                wk_sb[:, k, 128 * ot:128 * ot + 128],
                                xt_sb[:, k, 512 * jj:512 * jj + 512],
                                start=(k == 0), stop=(k == NK - 1))
                    return cl

                def cp():
                    for jj in range(NJ):
                        sl = slice(512 * jj, 512 * jj + 512)
                        if jj % 2 == 0:
                            nc.vector.tensor_copy(kp_sb[:, ot, sl],
                                                  st_["kp"][jj][:, :])
                        else:
                            nc.scalar.copy(kp_sb[:, ot, sl],
                                           st_["kp"][jj][:, :])
                    for hh in range(2):
                        h = 2 * ot + hh
                        nc.sync.dma_start(
                            out=kaug[0:64, h, :],
                            in_=kp_sb[64 * hh:64 * hh + 64, ot, :])
                return [mmk(k) for k in range(NK)] + [cp]

            def qaug_closure():
                def cl():
                    for h in range(HPC):
                        pr, hh = divmod(h, 2)
                        nc.sync.dma_start(
                            out=qaug[0:64, h, :],
                            in_=qp_sb[64 * hh:64 * hh + 64, pr, :])
                return [cl]

            # Emission below is organized as lists of closures ("streams")
            # that are proportionally interleaved: pass2(j) (ACT-paced) is
            # merged instruction-by-instruction with pass1(j+1) (DVE-paced),
            # vproj(j+1) and outproj(j-1) (PE-pure / mixed) so that every
            # engine has ready work throughout the chunk.

            def vproj_closures(st):
                st_ = {}

                def c1():
                    pv = ps.tile([P, OC], f32, tag="proj", bufs=2, name="pv")
                    st_["pv"] = pv
                    for k in range(4):
                        nc.tensor.matmul(
                            pv[:, :], xt_sb[:, k, P * st:P * st + P],
                            wv_sb[:, k, :], start=(k == 0), stop=False)

                def c2():
                    pv = st_["pv"]
                    for k in range(4, NK):
                        nc.tensor.matmul(
                            pv[:, :], xt_sb[:, k, P * st:P * st + P],
                            wv_sb[:, k, :], start=False, stop=(k == NK - 1))

                def c3():
                    pv4 = st_["pv"][:, :].rearrange("p (g x d) -> p g x d",
                                                    g=2, x=2)
                    nc.scalar.copy(vv0[:, st, :, 0:64], pv4[:, :, 0, :])
                    nc.scalar.copy(vv1[:, st, :, 64:P], pv4[:, :, 1, :])

                return [c1, c2, c3]

            def pass1_closures(pr, t):
                # scores [sq, sk] for one sq-tile, 2-head tile-packed; fused
                # (+causal mask) -> rowmax into m2 columns.
                ncols = (t + 1) * P
                nch = (ncols + 511) // 512
                st_ = {}

                def mk(c):
                    def cl():
                        n = min(512, ncols - 512 * c)
                        sa = ps.tile([P, 512], f32, tag="s1", bufs=2, name="sa")
                        sb_ = ps.tile([P, 512], f32, tag="s1", bufs=2,
                                      name="sb_")
                        last = c == nch - 1
                        if c == 0 and nch > 1:
                            st_["m4a"] = tmp.tile([P, 4], f32, tag="m4a",
                                                  bufs=2, name="m4a")
                            st_["m4b"] = tmp.tile([P, 4], f32, tag="m4b",
                                                  bufs=2, name="m4b")
                        nc.tensor.matmul(
                            sa[:, :n], qp_sb[0:64, pr, P * t:P * t + P],
                            kp_sb[0:64, pr, 512 * c:512 * c + n],
                            start=True, stop=not last, tile_position=(0, 0))
                        nc.tensor.matmul(
                            sb_[:, :n], qp_sb[64:P, pr, P * t:P * t + P],
                            kp_sb[64:P, pr, 512 * c:512 * c + n],
                            start=True, stop=not last, tile_position=(64, 0))
                        if last:
                            doff = n - P
                            nc.tensor.matmul(sa[:, doff:doff + P],
                                             ident16[:, :], cmask16[:, :],
                                             start=False, stop=True)
                            nc.tensor.matmul(sb_[:, doff:doff + P],
                                             ident16[:, :], cmask16[:, :],
                                             start=False, stop=True)
                        if nch == 1:
                            nc.vector.reduce_max(m2[:, pr, t:t + 1], sa[:, :n],
                                                 axis=AX)
                            nc.vector.reduce_max(m2[:, pr, 16 + t:16 + t + 1],
                                                 sb_[:, :n], axis=AX)
                        else:
                            m4a, m4b = st_["m4a"], st_["m4b"]
                            nc.vector.reduce_max(m4a[:, c:c + 1], sa[:, :n],
                                                 axis=AX)
                            nc.vector.reduce_max(m4b[:, c:c + 1], sb_[:, :n],
                                                 axis=AX)
                            if last:
                                nc.vector.reduce_max(m2[:, pr, t:t + 1],
                                                     m4a[:, 0:nch], axis=AX)
                                nc.vector.reduce_max(
                                    m2[:, pr, 16 + t:16 + t + 1],
                                    m4b[:, 0:nch], axis=AX)
                    return cl

                return [mk(c) for c in range(nch)]

            def mrow_closure(pr, j):
                # -M for chunk j's four sq-tiles -> row 64 of qaug, per head.
                def cl():
                    for hh in range(2):
                        mt_ps = ps.tile([4, P], f32, tag="s1", bufs=2,
                                        name="mt_ps")
                        nc.tensor.transpose(
                            mt_ps[:, :],
                            m2[:, pr, 16 * hh + 4 * j:16 * hh + 4 * j + 4],
                            ident[:, :])
                        mt_t = tmp.tile([4, P], f16, tag="mt", bufs=2,
                                        name="mt_t")
                        nc.scalar.activation(mt_t[:, :], mt_ps[:, :], COPY,
                                             scale=-1.0)
                        nc.sync.dma_start(
                            out=qaug[64:65, 2 * pr + hh,
                                     512 * j:512 * j + 512].rearrange(
                                         "q (t p) -> q t p", t=4),
                            in_=mt_t[:, :])
                return [cl]

            def pass2_closures(h, j):
                # scoresT with folded -M (and -30000 below-diagonal mask
                # accumulated on the PE for diagonal tiles, so exp gives
                # exact zeros there with no cross-engine select), exp into
                # fp16, and the ctx matmul skewed two tiles behind its exp
                # so the PE never waits on the ACT pipeline tail.
                pr, hh = divmod(h, 2)
                nt_here = 4 * j + 4
                st_ = {"ctxq": []}

                def emit_ctx(force=False):
                    depth = 0 if force else 2
                    while len(st_["ctxq"]) > depth:
                        t, qoff, n, pt = st_["ctxq"].pop(0)
                        lhsT = (vv0[:, t, pr, :] if hh == 0
                                else vv1[:, t, pr, :])
                        nc.tensor.matmul(
                            st_["ctxp"][0:(65 if hh == 0 else P),
                                        qoff - 512 * j:qoff - 512 * j + n],
                            lhsT, pt[:, :n],
                            start=(t == 0), stop=(t == nt_here - 1))

                def mk(t):
                    def cl():
                        if t == 0:
                            st_["ctxp"] = ps.tile([P, 512], f32, tag="ctx",
                                                  bufs=2, name="ctxp")
                        if t < 4 * j:
                            qoff, n = 512 * j, 512
                        else:
                            qoff = P * t
                            n = 512 * j + 512 - P * t
                        s2p = ps.tile([P, 512], f32, tag="s2", bufs=2,
                                      name="s2p")
                        diag = t >= 4 * j
                        nc.tensor.matmul(
                            s2p[:, :n], kaug[:, h, P * t:P * t + P],
                            qaug[:, h, qoff:qoff + n], start=True,
                            stop=not diag)
                        if diag:
                            nc.tensor.matmul(s2p[:, 0:P], ident16[:, :],
                                             cml16[:, :], start=False,
                                             stop=True)
                        pt = strm.tile([P, 512], f16, tag="pt", bufs=6,
                                       name="pt")
                        nc.scalar.activation(pt[:, :n], s2p[:, :n], EXP,
                                             scale=8.0)
                        st_["ctxq"].append((t, qoff, n, pt))
                        emit_ctx()
                    return cl

                def norm():
                    emit_ctx(force=True)
                    # normalize: ctx / rowsum
                    ctxp = st_["ctxp"]
                    rsrow = 64 if hh == 0 else 0
                    rr = tmp.tile([65, 512], f32, tag="rr", bufs=2, name="rr")
                    nc.vector.reciprocal(rr[rsrow:rsrow + 1, :],
                                         ctxp[rsrow:rsrow + 1, :])
                    rb = tmp.tile([P, 512], f32, tag="rb", bufs=2, name="rb")
                    nc.sync.dma_start(
                        out=rb[64 * hh:64 * hh + 64, :],
                        in_=rr[rsrow:rsrow + 1, :].unsqueeze(1).broadcast_to(
                            (1, 64, 512)))
                    nc.vector.tensor_mul(
                        ctxn[64 * hh:64 * hh + 64, pr, 512 * j:512 * j + 512],
                        ctxp[64 * hh:64 * hh + 64, :],
                        rb[64 * hh:64 * hh + 64, :])

                return [mk(t) for t in range(nt_here)] + [norm]

            outT_v = outT[:].rearrange("(o p) s -> p o s", p=P)

            def outproj_closures(j):
                st_ = {}

                def mk(oo):
                    def cl():
                        if oo == 0:
                            st_["ob"] = strm.tile([P, NK, 512], f16, tag="ob",
                                                  bufs=2, name="ob")
                        ob = st_["ob"]
                        po = ps.tile([P, 512], f32, tag="proj", bufs=2,
                                     name="po")
                        for kt in range(2):
                            nc.tensor.matmul(
                                po[:, :], wo_sb[:, kt, P * oo:P * oo + P],
                                ctxn[:, kt, 512 * j:512 * j + 512],
                                start=(kt == 0), stop=(kt == 1))
                        if oo % 2 == 0:
                            nc.scalar.copy(ob[:, oo, :], po[:, :])
                        else:
                            nc.vector.tensor_copy(ob[:, oo, :], po[:, :])
                        if oo == 3:
                            nc.sync.dma_start(
                                out=outT_v[:, 0:4, 512 * j:512 * j + 512],
                                in_=ob[:, 0:4, :])
                        elif oo == 7:
                            nc.sync.dma_start(
                                out=outT_v[:, 4:NK, 512 * j:512 * j + 512],
                                in_=ob[:, 4:NK, :])
                    return cl

                return [mk(oo) for oo in range(E // P)]

            def merge_emit(a_ops, b_ops, lead_b=3):
                # Proportionally interleave b_ops into a_ops (at most one
                # b per a so dep-blocked matmuls never pile up past the
                # 4-deep engine wait queue); all b_ops are drained before
                # returning, which callers rely on for cross-stream deps.
                bq = list(b_ops)
                if not a_ops:
                    for b in bq:
                        b()
                    return
                for _ in range(min(lead_b, len(bq))):
                    bq.pop(0)()
                ratio = len(bq) / len(a_ops)
                acc = 0.0
                for a in a_ops:
                    a()
                    acc += ratio
                    if acc >= 1.0 and bq:
                        bq.pop(0)()
                        acc -= 1.0
                for b in bq:
                    b()

            # ---- software-pipelined attention ----
            # k projection pr0 wave, then the DVE reduce train starts with
            # pass1(pr0, j=0) while the pr1 k-wave / vproj / aug DMAs fill PE.
            for cl in kproj_wave_closures(0, ["s1", "s1", "s2", "s2"]):
                cl()
            for cl in qaug_closure():
                cl()
            a_lead = []
            for t in range(4):
                a_lead += pass1_closures(0, t)
            a_lead += mrow_closure(0, 0)
            b_lead = kproj_wave_closures(1, ["ctx", "ctx", "proj", "proj"])
            for st in range(4):
                b_lead += vproj_closures(st)
            merge_emit(a_lead, b_lead)
            for j in range(NJ):
                # pr0 heads merge against [pass1(pr1, j) + mrow(pr1, j)]
                # (which MUST fully emit before the pr1 heads' pass2 so the
                # -M row is written before it is read), pr1 heads against
                # the rest of the next window's feed work.
                a0 = pass2_closures(0, j) + pass2_closures(1, j)
                a1 = pass2_closures(2, j) + pass2_closures(3, j)
                b0 = []
                for t in range(4 * j, 4 * j + 4):
                    b0 += pass1_closures(1, t)
                b0 += mrow_closure(1, j)
                b1 = []
                if j > 0:
                    b1 += outproj_closures(j - 1)
                if j < NJ - 1:
                    for st in range(4 * (j + 1), 4 * (j + 1) + 4):
                        b1 += vproj_closures(st)
                    for t in range(4 * (j + 1), 4 * (j + 1) + 4):
                        b1 += pass1_closures(0, t)
                    b1 += mrow_closure(0, j + 1)
                half = len(b1) // 2
                merge_emit(a0, b0 + b1[:half])
                merge_emit(a1, b1[half:])
            for cl in outproj_closures(NJ - 1):
                cl()

    nc.compile()
    return nc


def _get_nc():
    if "nc" not in _CACHE:
        _CACHE["nc"] = _build()
    return _CACHE["nc"]


def _make_cached_runner(nc):
    """Trace/compile the 8-core PJRT executable once; reuse on later calls.

    Mirrors concourse.bass2jax.run_bass_via_pjrt's multi-core branch, but
    keeps the jitted shard_map so repeat kernel() calls skip re-trace and
    re-lowering (the NEFF itself is already cached by neuronx_cc_hook).
    """
    import jax
    import jax.numpy as jnp
    from jax.sharding import Mesh, PartitionSpec
    from jax.experimental.shard_map import shard_map
    from concourse import bass2jax, mybir

    bass2jax.install_neuronx_cc_hook()
    partition_name = nc.partition_id_tensor.name if nc.partition_id_tensor else None
    in_names, out_names, out_avals = [], [], []
    for alloc in nc.m.functions[0].allocations:
        if not isinstance(alloc, mybir.MemoryLocationSet):
            continue
        name = alloc.memorylocations[0].name
        if alloc.kind == "ExternalInput":
            if name != partition_name:
                in_names.append(name)
        elif alloc.kind == "ExternalOutput":
            out_names.append(name)
            out_avals.append(jax.core.ShapedArray(
                tuple(alloc.tensor_shape), mybir.dt.np(alloc.dtype)))
    n_params = len(in_names)
    n_outs = len(out_avals)
    all_names = list(in_names) + list(out_names)
    if partition_name is not None:
        all_names.append(partition_name)

    def _body(*args):
        operands = list(args)
        if partition_name is not None:
            operands.append(bass2jax.partition_id_tensor())
        outs = bass2jax._bass_exec_p.bind(
            *operands,
            out_avals=tuple(out_avals),
            in_names=tuple(all_names),
            out_names=tuple(out_names),
            lowering_input_output_aliases=(),
            sim_require_finite=True,
            sim_require_nnan=True,
            nc=nc,
        )
        return tuple(outs)

    devices = jax.devices()[:NCORES]
    mesh = Mesh(np.asarray(devices), ("core",))
    in_specs = (PartitionSpec("core"),) * (n_params + n_outs)
    out_specs = (PartitionSpec("core"),) * n_outs
    sharded = jax.jit(
        shard_map(_body, mesh=mesh, in_specs=in_specs, out_specs=out_specs,
                  check_rep=False),
        donate_argnums=tuple(range(n_params, n_params + n_outs)),
        keep_unused=True)

    def run(in_maps):
        concat_in = [
            np.concatenate([np.asarray(in_maps[c][nm]) for c in range(NCORES)],
                           axis=0)
            for nm in in_names]
        concat_zeros = [
            np.zeros((NCORES * a.shape[0], *a.shape[1:]), a.dtype)
            for a in out_avals]
        out_arrs = sharded(*concat_in, *concat_zeros)
        return [
            {nm: np.asarray(out_arrs[i]).reshape(NCORES, *out_avals[i].shape)[c]
             for i, nm in enumerate(out_names)}
            for c in range(NCORES)]

    return run


def kernel(x, Wq, Wk, Wv, Wo):
    from concourse.bass_utils import run_bass_kernel_spmd

    # Force host numpy immediately: if the caller hands us jax arrays, any
    # .astype/.T on them would dispatch tiny jit programs to the neuron
    # backend, which wedges the device (known neuron-jit crash path).
    x, Wq, Wk, Wv, Wo = (np.asarray(a) for a in (x, Wq, Wk, Wv, Wo))

    nc = _get_nc()
    x16 = np.ascontiguousarray(x.astype(np.float16))
    Wq16 = Wq.astype(np.float16)
    Wk16 = Wk.astype(np.float16)
    Wv16 = Wv.astype(np.float16)
    Wo16 = Wo.astype(np.float16)

    xTs = [np.ascontiguousarray(x16[b].T) for b in range(B)]
    in_maps = []
    for c in range(NCORES):
        b, hg = divmod(c, HG)
        hsl = slice(OC * hg, OC * hg + OC)
        in_maps.append({
            "xt": xTs[b],
            "wq": np.ascontiguousarray(Wq16[hsl, :].T),
            "wk": np.ascontiguousarray(Wk16[hsl, :].T),
            "wv": np.ascontiguousarray(Wv16[hsl, :].T),
            "wo": np.ascontiguousarray(Wo16[:, hsl].T),
        })

    if "runner" in _CACHE:
        results = _CACHE["runner"](in_maps)
    else:
        # first call: compile + run through the sanctioned entry point,
        # then build the cached executable for subsequent calls
        results = run_bass_kernel_spmd(nc, in_maps, list(range(NCORES))).results
        _CACHE["runner"] = _make_cached_runner(nc)

    out = np.zeros((B, S, E), np.float32)
    for c in range(NCORES):
        b = c // HG
        out[b] += results[c]["outT"].T.astype(np.float32)
    return out


# revision 17
# speedup vs baseline: 1.1668x; 1.0285x over previous
"""Trainium2 Bass kernel for nn_AutoregressiveSelfAttention.

Sharding (8 cores): batch (2-way) x head-group (4-way tensor parallel).
Core c: batch c//4, heads [4*(c%4), 4*(c%4)+4).
Per-core: fp16 matmuls throughout (QKV proj, scores, P@V, out-proj),
fp32 softmax statistics, fp16 partial output; host sums the 4 head-group
partials per batch (the row-parallel all-reduce) in fp32 and transposes.

Softmax without transposes: pass1 computes scores [sq, sk] only to get the
causal row-max M (fused mask+max on DVE); pass2 recomputes scores
transposed with the max-subtraction folded in as a rank-1 matmul term
([kT;1].T @ [qT;-M]), exps on ACT into fp16, and the ctx matmul against
[v|1] accumulates both ctx and the softmax denominator in one PSUM tile.

v2 restructure vs baseline:
- q/k projections run contraction-outer across all 8 PSUM banks so the
  first matmuls start as soon as the first xt chunk lands (DMA-paced).
- Input DMAs batched per tensor (wq, wk, xt x8, wv, wo); augmented q/k
  copies batched to one [64, S] DMA per (tensor, head) issued upfront.
- Engine rebalance: projection PSUM->SBUF copies and out-proj copies on
  ACT (was DVE), -M negate folded into an ACT copy (scale=-1), DVE keeps
  only the row-max reduces, reciprocals and normalize multiplies.
- Emission order software-pipelines across chunks: pass1(pr1) hides
  mrow(pr0) DMA latency, vproj(j+1) hides normalize latency before
  outproj(j).
- Output stored fp16 and written with one batched DMA per seq chunk.
"""
import sys
sys.path.insert(0, "/opt/trn_rl_repo")

import math
import numpy as np

B, S, E, H, D = 2, 2048, 1024, 16, 64
NCORES = 8
HG = 4                  # head-group shards
HPC = H // HG           # 4 heads per core
OC = HPC * D            # 256 per-core projection width
P = 128
NK = E // P             # 8 contraction tiles for projections
NT = S // P             # 16 seq tiles of 128
NJ = S // 512           # 4 seq chunks of 512

_CACHE = {}


def _build():
    import concourse.bacc as bacc
    import concourse.mybir as mybir
    import concourse.tile as tile
    from concourse.masks import make_identity, make_causal_mask

    dt = mybir.dt
    f32, f16 = dt.float32, dt.float16
    AX = mybir.AxisListType.X
    ALU = mybir.AluOpType
    COPY = mybir.ActivationFunctionType.Copy
    EXP = mybir.ActivationFunctionType.Exp

    nc = bacc.Bacc(None, target_bir_lowering=False, debug=False)
    with tile.TileContext(nc) as tc:
        with tc.tile_pool(name="dram", bufs=1, space="DRAM") as dram, \
             tc.tile_pool(name="persist", bufs=1) as pers, \
             tc.tile_pool(name="stream", bufs=4) as strm, \
             tc.tile_pool(name="tmp", bufs=4) as tmp, \
             tc.tile_pool(name="ps", bufs=1, space="PSUM") as ps:

            # ---- DRAM I/O ----
            xt = dram.tile([E, S], f16, kind="ExternalInput", name="xt", uniquify=False)
            wq = dram.tile([E, OC], f16, kind="ExternalInput", name="wq", uniquify=False)
            wk = dram.tile([E, OC], f16, kind="ExternalInput", name="wk", uniquify=False)
            wv = dram.tile([E, OC], f16, kind="ExternalInput", name="wv", uniquify=False)
            wo = dram.tile([OC, E], f16, kind="ExternalInput", name="wo", uniquify=False)
            outT = dram.tile([E, S], f16, kind="ExternalOutput", name="outT", uniquify=False)

            # ---- persistent SBUF ----
            xt_sb = pers.tile([P, NK, S], f16)
            wq_sb = pers.tile([P, NK, OC], f16)
            wk_sb = pers.tile([P, NK, OC], f16)
            wv_sb = pers.tile([P, NK, OC], f16)
            wo_sb = pers.tile([P, 2, E], f16)
            qp_sb = pers.tile([P, 2, S], f16)       # qT, head-pair stacked
            kp_sb = pers.tile([P, 2, S], f16)       # kT, head-pair stacked
            qaug = pers.tile([65, HPC, S], f16)     # [qT_h ; -M_h] per head
            kaug = pers.tile([65, HPC, S], f16)     # [kT_h ; ones] per head
            vv0 = pers.tile([P, NT, 2, 65], f16)    # heads 0,2: [v(0:64) | ones]
            vv1 = pers.tile([P, NT, 2, P], f16)     # heads 1,3: [ones|0*63|v(64:128)]
            ctxn = pers.tile([P, 2, S], f16)        # normalized ctx, pair stacked
            m2 = pers.tile([P, 2, 32], f32)         # rowmax cols per pair (hh*16+t)
            ident = pers.tile([P, P], f32)
            ident16 = pers.tile([P, P], f16)
            cmask16 = pers.tile([P, P], f16)        # 0 / -30000 above diag
            cml16 = pers.tile([P, P], f16)          # 0 / -30000 below diag

            # ---- input DMAs (batched, consumer order) ----
            xt_v = xt[:].rearrange("(k p) s -> p k s", p=P)
            nc.sync.dma_start(out=wq_sb[:, :, :],
                              in_=wq[:].rearrange("(k p) o -> p k o", p=P))
            for k in range(NK):
                nc.sync.dma_start(out=xt_sb[:, k, :], in_=xt_v[:, k, :])
            nc.sync.dma_start(out=wk_sb[:, :, :],
                              in_=wk[:].rearrange("(k p) o -> p k o", p=P))
            nc.sync.dma_start(out=wv_sb[:, :, :],
                              in_=wv[:].rearrange("(k p) o -> p k o", p=P))
            nc.sync.dma_start(out=wo_sb[:, :, :],
                              in_=wo[:].rearrange("(k p) e -> p k e", p=P))

            # ---- constants ----
            make_identity(nc, ident[:, :])
            make_identity(nc, ident16[:, :])
            make_causal_mask(nc, cmask16[:, :], mask_val=-30000.0)
            # cml16: -30000 strictly below the diagonal (masks k > q in the
            # transposed scores), built by affine-filling a zero tile.
            nc.gpsimd.memset(cml16[:, :], 0.0)
            nc.gpsimd.affine_select(
                out=cml16[:, :], in_=cml16[:, :],
                compare_op=ALU.is_ge, fill=-30000.0, base=0,
                pattern=[[1, P]], channel_multiplier=-1)
            nc.gpsimd.memset(kaug[64:65, :, :], 1.0)
            nc.gpsimd.memset(vv0[:, :, :, 64:65], 1.0)
            nc.gpsimd.memset(vv1[:, :, :, 0:1], 1.0)
            nc.gpsimd.memset(vv1[:, :, :, 1:64], 0.0)

            # ---- q projection: contraction-outer over all 8 PSUM banks so
            # matmuls pace with the xt chunk DMAs ----
            TAGS8 = ["s1", "s1", "s2", "s2", "ctx", "ctx", "proj", "proj"]
            pp = []
            for i in range(8):
                t_ = ps.tile([P, 512], f32, tag=TAGS8[i], bufs=2,
                             name=f"pp{i}")
                pp.append(t_)
            for k in range(NK):
                for i in range(8):
                    ot, j = divmod(i, NJ)
                    nc.tensor.matmul(
                        pp[i][:, :],
                        wq_sb[:, k, 128 * ot:128 * ot + 128],
                        xt_sb[:, k, 512 * j:512 * j + 512],
                        start=(k == 0), stop=(k == NK - 1))
            for i in range(8):
                ot, j = divmod(i, NJ)
                sl = slice(512 * j, 512 * j + 512)
                if i % 2 == 0:
                    nc.vector.tensor_copy(qp_sb[:, ot, sl], pp[i][:, :])
                else:
                    nc.scalar.copy(qp_sb[:, ot, sl], pp[i][:, :])

            def kproj_wave_closures(ot, tags):
                # one head-pair of the k projection: 4 tiles, k-outer
                st_ = {}

                def alloc():
                    st_["kp"] = [ps.tile([P, 512], f32, tag=tags[i], bufs=2,
                                         name=f"kpp{i}") for i in range(4)]

                def mmk(k):
                    def cl():
                        if k == 0:
                            alloc()
                        for jj in range(NJ):
                            nc.tensor.matmul(
                                st_["kp"][jj][:, :],
                                wk_sb[:, k, 128 * ot:128 * ot + 128],
                                xt_sb[:, k, 512 * jj:512 * jj + 512],
                                start=(k == 0), stop=(k == NK - 1))
                    return cl

                def cp():
                    for jj in range(NJ):
                        sl = slice(512 * jj, 512 * jj + 512)
                        if jj % 2 == 0:
                            nc.vector.tensor_copy(kp_sb[:, ot, sl],
                                                  st_["kp"][jj][:, :])
                        else:
                            nc.scalar.copy(kp_sb[:, ot, sl],
                                           st_["kp"][jj][:, :])
                    for hh in range(2):
                        h = 2 * ot + hh
                        nc.sync.dma_start(
                            out=kaug[0:64, h, :],
                            in_=kp_sb[64 * hh:64 * hh + 64, ot, :])
                return [mmk(k) for k in range(NK)] + [cp]

            def qaug_closure():
                def cl():
                    for h in range(HPC):
                        pr, hh = divmod(h, 2)
                        nc.sync.dma_start(
                            out=qaug[0:64, h, :],
                            in_=qp_sb[64 * hh:64 * hh + 64, pr, :])
                return [cl]

            # Emission below is organized as lists of closures ("streams")
            # that are proportionally interleaved: pass2(j) (ACT-paced) is
            # merged instruction-by-instruction with pass1(j+1) (DVE-paced),
            # vproj(j+1) and outproj(j-1) (PE-pure / mixed) so that every
            # engine has ready work throughout the chunk.

            def vproj_closures(st):
                st_ = {}

                def c1():
                    pv = ps.tile([P, OC], f32, tag="proj", bufs=2, name="pv")
                    st_["pv"] = pv
                    for k in range(4):
                        nc.tensor.matmul(
                            pv[:, :], xt_sb[:, k, P * st:P * st + P],
                            wv_sb[:, k, :], start=(k == 0), stop=False)

                def c2():
                    pv = st_["pv"]
                    for k in range(4, NK):
                        nc.tensor.matmul(
                            pv[:, :], xt_sb[:, k, P * st:P * st + P],
                            wv_sb[:, k, :], start=False, stop=(k == NK - 1))

                def c3():
                    pv4 = st_["pv"][:, :].rearrange("p (g x d) -> p g x d",
                                                    g=2, x=2)
                    nc.scalar.copy(vv0[:, st, :, 0:64], pv4[:, :, 0, :])
                    nc.scalar.copy(vv1[:, st, :, 64:P], pv4[:, :, 1, :])

                return [c1, c2, c3]

            def pass1_closures(pr, t):
                # scores [sq, sk] for one sq-tile, 2-head tile-packed; fused
                # (+causal mask) -> rowmax into m2 columns.
                ncols = (t + 1) * P
                nch = (ncols + 511) // 512
                st_ = {}

                def mk(c):
                    def cl():
                        n = min(512, ncols - 512 * c)
                        sa = ps.tile([P, 512], f32, tag="s1", bufs=2, name="sa")
                        sb_ = ps.tile([P, 512], f32, tag="s1", bufs=2,
                                      name="sb_")
                        last = c == nch - 1
                        if c == 0 and nch > 1:
                            st_["m4a"] = tmp.tile([P, 4], f32, tag="m4a",
                                                  bufs=2, name="m4a")
                            st_["m4b"] = tmp.tile([P, 4], f32, tag="m4b",
                                                  bufs=2, name="m4b")
                        nc.tensor.matmul(
                            sa[:, :n], qp_sb[0:64, pr, P * t:P * t + P],
                            kp_sb[0:64, pr, 512 * c:512 * c + n],
                            start=True, stop=not last, tile_position=(0, 0))
                        nc.tensor.matmul(
                            sb_[:, :n], qp_sb[64:P, pr, P * t:P * t + P],
                            kp_sb[64:P, pr, 512 * c:512 * c + n],
                            start=True, stop=not last, tile_position=(64, 0))
                        if last:
                            doff = n - P
                            nc.tensor.matmul(sa[:, doff:doff + P],
                                             ident16[:, :], cmask16[:, :],
                                             start=False, stop=True)
                            nc.tensor.matmul(sb_[:, doff:doff + P],
                                             ident16[:, :], cmask16[:, :],
                                             start=False, stop=True)
                        if nch == 1:
                            nc.vector.reduce_max(m2[:, pr, t:t + 1], sa[:, :n],
                                                 axis=AX)
                            nc.vector.reduce_max(m2[:, pr, 16 + t:16 + t + 1],
                                                 sb_[:, :n], axis=AX)
                        else:
                            m4a, m4b = st_["m4a"], st_["m4b"]
                            nc.vector.reduce_max(m4a[:, c:c + 1], sa[:, :n],
                                                 axis=AX)
                            nc.vector.reduce_max(m4b[:, c:c + 1], sb_[:, :n],
                                                 axis=AX)
                            if last:
                                nc.vector.reduce_max(m2[:, pr, t:t + 1],
                                                     m4a[:, 0:nch], axis=AX)
                                nc.vector.reduce_max(
                                    m2[:, pr, 16 + t:16 + t + 1],
                                    m4b[:, 0:nch], axis=AX)
                    return cl

                return [mk(c) for c in range(nch)]

            def mrow_closure(pr, j):
                # -M for chunk j's four sq-tiles -> row 64 of qaug, per head.
                def cl():
                    for hh in range(2):
                        mt_ps = ps.tile([4, P], f32, tag="s1", bufs=2,
                                        name="mt_ps")
                        nc.tensor.transpose(
                            mt_ps[:, :],
                            m2[:, pr, 16 * hh + 4 * j:16 * hh + 4 * j + 4],
                            ident[:, :])
                        mt_t = tmp.tile([4, P], f16, tag="mt", bufs=2,
                                        name="mt_t")
                        nc.scalar.activation(mt_t[:, :], mt_ps[:, :], COPY,
                                             scale=-1.0)
                        nc.sync.dma_start(
                            out=qaug[64:65, 2 * pr + hh,
                                     512 * j:512 * j + 512].rearrange(
                                         "q (t p) -> q t p", t=4),
                            in_=mt_t[:, :])
                return [cl]

            def pass2_closures(h, j):
                # scoresT with folded -M (and -30000 below-diagonal mask
                # accumulated on the PE for diagonal tiles, so exp gives
                # exact zeros there with no cross-engine select), exp into
                # fp16, and the ctx matmul skewed two tiles behind its exp
                # so the PE never waits on the ACT pipeline tail.
                pr, hh = divmod(h, 2)
                nt_here = 4 * j + 4
                st_ = {"ctxq": []}

                def emit_ctx(force=False):
                    depth = 0 if force else 2
                    while len(st_["ctxq"]) > depth:
                        t, qoff, n, pt = st_["ctxq"].pop(0)
                        lhsT = (vv0[:, t, pr, :] if hh == 0
                                else vv1[:, t, pr, :])
                        nc.tensor.matmul(
                            st_["ctxp"][0:(65 if hh == 0 else P),
                                        qoff - 512 * j:qoff - 512 * j + n],
                            lhsT, pt[:, :n],
                            start=(t == 0), stop=(t == nt_here - 1))

                def mk(t):
                    def cl():
                        if t == 0:
                            st_["ctxp"] = ps.tile([P, 512], f32, tag="ctx",
                                                  bufs=2, name="ctxp")
                        if t < 4 * j:
                            qoff, n = 512 * j, 512
                        else:
                            qoff = P * t
                            n = 512 * j + 512 - P * t
                        s2p = ps.tile([P, 512], f32, tag="s2", bufs=2,
                                      name="s2p")
                        nc.tensor.matmul(
                            s2p[:, :n], kaug[:, h, P * t:P * t + P],
                            qaug[:, h, qoff:qoff + n], start=True, stop=True)
                        pt = strm.tile([P, 512], f16, tag="pt", bufs=6,
                                       name="pt")
                        nc.scalar.activation(pt[:, :n], s2p[:, :n], EXP,
                                             scale=8.0)
                        if t >= 4 * j:
                            # zero strictly-upper block at the diagonal (the
                            # ctx skew hides the Pool round-trip)
                            nc.gpsimd.affine_select(
                                out=pt[:, 0:P], in_=pt[:, 0:P],
                                compare_op=ALU.is_ge, fill=0.0, base=0,
                                pattern=[[1, P]], channel_multiplier=-1)
                        st_["ctxq"].append((t, qoff, n, pt))
                        emit_ctx()
                    return cl

                def norm():
                    emit_ctx(force=True)
                    # normalize: ctx / rowsum
                    ctxp = st_["ctxp"]
                    rsrow = 64 if hh == 0 else 0
                    rr = tmp.tile([65, 512], f32, tag="rr", bufs=2, name="rr")
                    nc.vector.reciprocal(rr[rsrow:rsrow + 1, :],
                                         ctxp[rsrow:rsrow + 1, :])
                    rb = tmp.tile([P, 512], f32, tag="rb", bufs=2, name="rb")
                    nc.sync.dma_start(
                        out=rb[64 * hh:64 * hh + 64, :],
                        in_=rr[rsrow:rsrow + 1, :].unsqueeze(1).broadcast_to(
                            (1, 64, 512)))
                    nc.vector.tensor_mul(
                        ctxn[64 * hh:64 * hh + 64, pr, 512 * j:512 * j + 512],
                        ctxp[64 * hh:64 * hh + 64, :],
                        rb[64 * hh:64 * hh + 64, :])

                return [mk(t) for t in range(nt_here)] + [norm]

            outT_v = outT[:].rearrange("(o p) s -> p o s", p=P)

            def outproj_closures(j, copy_dve=False):
                st_ = {}

                def mk(oo):
                    def cl():
                        if oo == 0:
                            st_["ob"] = strm.tile([P, NK, 512], f16, tag="ob",
                                                  bufs=2, name="ob")
                        ob = st_["ob"]
                        po = ps.tile([P, 512], f32, tag="proj", bufs=2,
                                     name="po")
                        for kt in range(2):
                            nc.tensor.matmul(
                                po[:, :], wo_sb[:, kt, P * oo:P * oo + P],
                                ctxn[:, kt, 512 * j:512 * j + 512],
                                start=(kt == 0), stop=(kt == 1))
                        if copy_dve:
                            nc.vector.tensor_copy(ob[:, oo, :], po[:, :])
                        else:
                            nc.scalar.copy(ob[:, oo, :], po[:, :])
                        if oo == 3:
                            nc.sync.dma_start(
                                out=outT_v[:, 0:4, 512 * j:512 * j + 512],
                                in_=ob[:, 0:4, :])
                        elif oo == 7:
                            nc.sync.dma_start(
                                out=outT_v[:, 4:NK, 512 * j:512 * j + 512],
                                in_=ob[:, 4:NK, :])
                    return cl

                return [mk(oo) for oo in range(E // P)]

            def merge_emit(a_ops, b_ops, lead_b=3):
                # Proportionally interleave b_ops into a_ops (at most one
                # b per a so dep-blocked matmuls never pile up past the
                # 4-deep engine wait queue); all b_ops are drained before
                # returning, which callers rely on for cross-stream deps.
                bq = list(b_ops)
                if not a_ops:
                    for b in bq:
                        b()
                    return
                for _ in range(min(lead_b, len(bq))):
                    bq.pop(0)()
                ratio = len(bq) / len(a_ops)
                acc = 0.0
                for a in a_ops:
                    a()
                    acc += ratio
                    if acc >= 1.0 and bq:
                        bq.pop(0)()
                        acc -= 1.0
                for b in bq:
                    b()

            # ---- software-pipelined attention, seq chunks processed in
            # DESCENDING order ----
            # The DVE row-max reduce train is the critical path; each chunk's
            # softmax (ACT exp) can only start after its own reduces, so the
            # train runs largest-consumer-first (j=3 ... 0) and the last
            # window leaves only the cheapest exp work + one out-proj after
            # the train drains (flow-shop tail minimization).
            for cl in kproj_wave_closures(0, ["s1", "s1", "s2", "s2"]):
                cl()
            for cl in qaug_closure():
                cl()
            a_lead = []
            for t in range(12, 16):
                a_lead += pass1_closures(0, t)
            a_lead += mrow_closure(0, 3)
            b_lead = kproj_wave_closures(1, ["ctx", "ctx", "proj", "proj"])
            for st in range(NT):
                b_lead += vproj_closures(st)
            merge_emit(a_lead, b_lead)
            for j in (3, 2, 1, 0):
                # pr0 heads merge against [pass1(pr1, j) + mrow(pr1, j)]
                # (fully drained before the pr1 heads' pass2 reads the -M
                # row), pr1 heads against the next train segment.
                a0 = pass2_closures(0, j) + pass2_closures(1, j)
                a1 = pass2_closures(2, j) + pass2_closures(3, j)
                b0 = []
                if j < 3:
                    b0 += outproj_closures(j + 1)
                for t in range(4 * j, 4 * j + 4):
                    b0 += pass1_closures(1, t)
                b0 += mrow_closure(1, j)
                b1 = []
                if j > 0:
                    for t in range(4 * (j - 1), 4 * (j - 1) + 4):
                        b1 += pass1_closures(0, t)
                    b1 += mrow_closure(0, j - 1)
                half = len(b1) // 2
                merge_emit(a0, b0 + b1[:half])
                merge_emit(a1, b1[half:])
            for cl in outproj_closures(0, copy_dve=True):
                cl()

    nc.compile()
    return nc


def _get_nc():
    if "nc" not in _CACHE:
        _CACHE["nc"] = _build()
    return _CACHE["nc"]


def _make_cached_runner(nc):
    """Trace/compile the 8-core PJRT executable once; reuse on later calls.

    Mirrors concourse.bass2jax.run_bass_via_pjrt's multi-core branch, but
    keeps the jitted shard_map so repeat kernel() calls skip re-trace and
    re-lowering (the NEFF itself is already cached by neuronx_cc_hook).
    """
    import jax
    import jax.numpy as jnp
    from jax.sharding import Mesh, PartitionSpec
    from jax.experimental.shard_map import shard_map
    from concourse import bass2jax, mybir

    bass2jax.install_neuronx_cc_hook()
    partition_name = nc.partition_id_tensor.name if nc.partition_id_tensor else None
    in_names, out_names, out_avals = [], [], []
    for alloc in nc.m.functions[0].allocations:
        if not isinstance(alloc, mybir.MemoryLocationSet):
            continue
        name = alloc.memorylocations[0].name
        if alloc.kind == "ExternalInput":
            if name != partition_name:
                in_names.append(name)
        elif alloc.kind == "ExternalOutput":
            out_names.append(name)
            out_avals.append(jax.core.ShapedArray(
                tuple(alloc.tensor_shape), mybir.dt.np(alloc.dtype)))
    n_params = len(in_names)
    n_outs = len(out_avals)
    all_names = list(in_names) + list(out_names)
    if partition_name is not None:
        all_names.append(partition_name)

    def _body(*args):
        operands = list(args)
        if partition_name is not None:
            operands.append(bass2jax.partition_id_tensor())
        outs = bass2jax._bass_exec_p.bind(
            *operands,
            out_avals=tuple(out_avals),
            in_names=tuple(all_names),
            out_names=tuple(out_names),
            lowering_input_output_aliases=(),
            sim_require_finite=True,
            sim_require_nnan=True,
            nc=nc,
        )
        return tuple(outs)

    devices = jax.devices()[:NCORES]
    mesh = Mesh(np.asarray(devices), ("core",))
    in_specs = (PartitionSpec("core"),) * (n_params + n_outs)
    out_specs = (PartitionSpec("core"),) * n_outs
    sharded = jax.jit(
        shard_map(_body, mesh=mesh, in_specs=in_specs, out_specs=out_specs,
                  check_rep=False),
        donate_argnums=tuple(range(n_params, n_params + n_outs)),
        keep_unused=True)

    def run(in_maps):
        concat_in = [
            np.concatenate([np.asarray(in_maps[c][nm]) for c in range(NCORES)],
                           axis=0)
            for nm in in_names]
        concat_zeros = [
            np.zeros((NCORES * a.shape[0], *a.shape[1:]), a.dtype)
            for a in out_avals]
        out_arrs = sharded(*concat_in, *concat_zeros)
        return [
            {nm: np.asarray(out_arrs[i]).reshape(NCORES, *out_avals[i].shape)[c]
             for i, nm in enumerate(out_names)}
            for c in range(NCORES)]

    return run


def kernel(x, Wq, Wk, Wv, Wo):
    from concourse.bass_utils import run_bass_kernel_spmd

    # Force host numpy immediately: if the caller hands us jax arrays, any
    # .astype/.T on them would dispatch tiny jit programs to the neuron
    # backend, which wedges the device (known neuron-jit crash path).
    x, Wq, Wk, Wv, Wo = (np.asarray(a) for a in (x, Wq, Wk, Wv, Wo))

    nc = _get_nc()
    x16 = np.ascontiguousarray(x.astype(np.float16))
    Wq16 = Wq.astype(np.float16)
    Wk16 = Wk.astype(np.float16)
    Wv16 = Wv.astype(np.float16)
    Wo16 = Wo.astype(np.float16)

    xTs = [np.ascontiguousarray(x16[b].T) for b in range(B)]
    in_maps = []
    for c in range(NCORES):
        b, hg = divmod(c, HG)
        hsl = slice(OC * hg, OC * hg + OC)
        in_maps.append({
            "xt": xTs[b],
            "wq": np.ascontiguousarray(Wq16[hsl, :].T),
            "wk": np.ascontiguousarray(Wk16[hsl, :].T),
            "wv": np.ascontiguousarray(Wv16[hsl, :].T),
            "wo": np.ascontiguousarray(Wo16[:, hsl].T),
        })

    if "runner" in _CACHE:
        results = _CACHE["runner"](in_maps)
    else:
        # first call: compile + run through the sanctioned entry point,
        # then build the cached executable for subsequent calls
        results = run_bass_kernel_spmd(nc, in_maps, list(range(NCORES))).results
        _CACHE["runner"] = _make_cached_runner(nc)

    out = np.zeros((B, S, E), np.float32)
    for c in range(NCORES):
        b = c // HG
        out[b] += results[c]["outT"].T.astype(np.float32)
    return out


# revision 18
# speedup vs baseline: 1.1831x; 1.0139x over previous
"""Trainium2 Bass kernel for nn_AutoregressiveSelfAttention.

Sharding (8 cores): batch (2-way) x head-group (4-way tensor parallel).
Core c: batch c//4, heads [4*(c%4), 4*(c%4)+4).
Per-core: fp16 matmuls throughout (QKV proj, scores, P@V, out-proj),
fp32 softmax statistics, fp16 partial output; host sums the 4 head-group
partials per batch (the row-parallel all-reduce) in fp32 and transposes.

Softmax without transposes: pass1 computes scores [sq, sk] only to get the
causal row-max M (fused mask+max on DVE); pass2 recomputes scores
transposed with the max-subtraction folded in as a rank-1 matmul term
([kT;1].T @ [qT;-M]), exps on ACT into fp16, and the ctx matmul against
[v|1] accumulates both ctx and the softmax denominator in one PSUM tile.

v2 restructure vs baseline:
- q/k projections run contraction-outer across all 8 PSUM banks so the
  first matmuls start as soon as the first xt chunk lands (DMA-paced).
- Input DMAs batched per tensor (wq, wk, xt x8, wv, wo); augmented q/k
  copies batched to one [64, S] DMA per (tensor, head) issued upfront.
- Engine rebalance: projection PSUM->SBUF copies and out-proj copies on
  ACT (was DVE), -M negate folded into an ACT copy (scale=-1), DVE keeps
  only the row-max reduces, reciprocals and normalize multiplies.
- Emission order software-pipelines across chunks: pass1(pr1) hides
  mrow(pr0) DMA latency, vproj(j+1) hides normalize latency before
  outproj(j).
- Output stored fp16 and written with one batched DMA per seq chunk.
"""
import sys
sys.path.insert(0, "/opt/trn_rl_repo")

import math
import numpy as np

B, S, E, H, D = 2, 2048, 1024, 16, 64
NCORES = 8
HG = 4                  # head-group shards
HPC = H // HG           # 4 heads per core
OC = HPC * D            # 256 per-core projection width
P = 128
NK = E // P             # 8 contraction tiles for projections
NT = S // P             # 16 seq tiles of 128
NJ = S // 512           # 4 seq chunks of 512

_CACHE = {}


def _build():
    import concourse.bacc as bacc
    import concourse.mybir as mybir
    import concourse.tile as tile
    from concourse.masks import make_identity, make_causal_mask

    dt = mybir.dt
    f32, f16 = dt.float32, dt.float16
    AX = mybir.AxisListType.X
    ALU = mybir.AluOpType
    COPY = mybir.ActivationFunctionType.Copy
    EXP = mybir.ActivationFunctionType.Exp

    nc = bacc.Bacc(None, target_bir_lowering=False, debug=False)
    with tile.TileContext(nc) as tc:
        with tc.tile_pool(name="dram", bufs=1, space="DRAM") as dram, \
             tc.tile_pool(name="persist", bufs=1) as pers, \
             tc.tile_pool(name="stream", bufs=4) as strm, \
             tc.tile_pool(name="tmp", bufs=4) as tmp, \
             tc.tile_pool(name="ps", bufs=1, space="PSUM") as ps:

            # ---- DRAM I/O ----
            xt = dram.tile([E, S], f16, kind="ExternalInput", name="xt", uniquify=False)
            wq = dram.tile([E, OC], f16, kind="ExternalInput", name="wq", uniquify=False)
            wk = dram.tile([E, OC], f16, kind="ExternalInput", name="wk", uniquify=False)
            wv = dram.tile([E, OC], f16, kind="ExternalInput", name="wv", uniquify=False)
            wo = dram.tile([OC, E], f16, kind="ExternalInput", name="wo", uniquify=False)
            outT = dram.tile([E, S], f16, kind="ExternalOutput", name="outT", uniquify=False)

            # ---- persistent SBUF ----
            xt_sb = pers.tile([P, NK, S], f16)
            wq_sb = pers.tile([P, NK, OC], f16)
            wk_sb = pers.tile([P, NK, OC], f16)
            wv_sb = pers.tile([P, NK, OC], f16)
            wo_sb = pers.tile([P, 2, E], f16)
            qp_sb = pers.tile([P, 2, S], f16)       # qT, head-pair stacked
            kp_sb = pers.tile([P, 2, S], f16)       # kT, head-pair stacked
            qaug = pers.tile([65, HPC, S], f16)     # [qT_h ; -M_h] per head
            kaug = pers.tile([65, HPC, S], f16)     # [kT_h ; ones] per head
            vv0 = pers.tile([P, NT, 2, 65], f16)    # heads 0,2: [v(0:64) | ones]
            vv1 = pers.tile([P, NT, 2, P], f16)     # heads 1,3: [ones|0*63|v(64:128)]
            ctxn = pers.tile([P, 2, S], f16)        # normalized ctx, pair stacked
            m2 = pers.tile([P, 2, 32], f32)         # rowmax cols per pair (hh*16+t)
            ident = pers.tile([P, P], f32)
            ident16 = pers.tile([P, P], f16)
            cmask16 = pers.tile([P, P], f16)        # 0 / -30000 above diag
            cml16 = pers.tile([P, P], f16)          # 0 / -30000 below diag

            # ---- input DMAs (batched, consumer order) ----
            xt_v = xt[:].rearrange("(k p) s -> p k s", p=P)
            nc.sync.dma_start(out=wq_sb[:, :, :],
                              in_=wq[:].rearrange("(k p) o -> p k o", p=P))
            for k in range(NK):
                nc.sync.dma_start(out=xt_sb[:, k, :], in_=xt_v[:, k, :])
            nc.sync.dma_start(out=wk_sb[:, :, :],
                              in_=wk[:].rearrange("(k p) o -> p k o", p=P))
            nc.sync.dma_start(out=wv_sb[:, :, :],
                              in_=wv[:].rearrange("(k p) o -> p k o", p=P))
            nc.sync.dma_start(out=wo_sb[:, :, :],
                              in_=wo[:].rearrange("(k p) e -> p k e", p=P))

            # ---- constants ----
            make_identity(nc, ident[:, :])
            make_identity(nc, ident16[:, :])
            make_causal_mask(nc, cmask16[:, :], mask_val=-30000.0)
            # cml16: -30000 strictly below the diagonal (masks k > q in the
            # transposed scores), built by affine-filling a zero tile.
            nc.gpsimd.memset(cml16[:, :], 0.0)
            nc.gpsimd.affine_select(
                out=cml16[:, :], in_=cml16[:, :],
                compare_op=ALU.is_ge, fill=-30000.0, base=0,
                pattern=[[1, P]], channel_multiplier=-1)
            nc.gpsimd.memset(kaug[64:65, :, :], 1.0)
            nc.gpsimd.memset(vv0[:, :, :, 64:65], 1.0)
            nc.gpsimd.memset(vv1[:, :, :, 0:1], 1.0)
            nc.gpsimd.memset(vv1[:, :, :, 1:64], 0.0)

            # ---- q projection: contraction-outer over all 8 PSUM banks so
            # matmuls pace with the xt chunk DMAs ----
            TAGS8 = ["s1", "s1", "s2", "s2", "ctx", "ctx", "proj", "proj"]
            pp = []
            for i in range(8):
                t_ = ps.tile([P, 512], f32, tag=TAGS8[i], bufs=2,
                             name=f"pp{i}")
                pp.append(t_)
            for k in range(NK):
                for i in range(8):
                    ot, j = divmod(i, NJ)
                    nc.tensor.matmul(
                        pp[i][:, :],
                        wq_sb[:, k, 128 * ot:128 * ot + 128],
                        xt_sb[:, k, 512 * j:512 * j + 512],
                        start=(k == 0), stop=(k == NK - 1))
            for i in range(8):
                ot, j = divmod(i, NJ)
                sl = slice(512 * j, 512 * j + 512)
                if i % 2 == 0:
                    nc.vector.tensor_copy(qp_sb[:, ot, sl], pp[i][:, :])
                else:
                    nc.scalar.copy(qp_sb[:, ot, sl], pp[i][:, :])

            def kproj_wave_closures(ot, tags):
                # one head-pair of the k projection: 4 tiles, k-outer
                st_ = {}

                def alloc():
                    st_["kp"] = [ps.tile([P, 512], f32, tag=tags[i], bufs=2,
                                         name=f"kpp{i}") for i in range(4)]

                def mmk(k):
                    def cl():
                        if k == 0:
                            alloc()
                        for jj in range(NJ):
                            nc.tensor.matmul(
                                st_["kp"][jj][:, :],
                                wk_sb[:, k, 128 * ot:128 * ot + 128],
                                xt_sb[:, k, 512 * jj:512 * jj + 512],
                                start=(k == 0), stop=(k == NK - 1))
                    return cl

                def cp():
                    for jj in range(NJ):
                        sl = slice(512 * jj, 512 * jj + 512)
                        if jj % 2 == 0:
                            nc.vector.tensor_copy(kp_sb[:, ot, sl],
                                                  st_["kp"][jj][:, :])
                        else:
                            nc.scalar.copy(kp_sb[:, ot, sl],
                                           st_["kp"][jj][:, :])
                    for hh in range(2):
                        h = 2 * ot + hh
                        nc.sync.dma_start(
                            out=kaug[0:64, h, :],
                            in_=kp_sb[64 * hh:64 * hh + 64, ot, :])
                return [mmk(k) for k in range(NK)] + [cp]

            def qaug_closure():
                def cl():
                    for h in range(HPC):
                        pr, hh = divmod(h, 2)
                        nc.sync.dma_start(
                            out=qaug[0:64, h, :],
                            in_=qp_sb[64 * hh:64 * hh + 64, pr, :])
                return [cl]

            # Emission below is organized as lists of closures ("streams")
            # that are proportionally interleaved: pass2(j) (ACT-paced) is
            # merged instruction-by-instruction with pass1(j+1) (DVE-paced),
            # vproj(j+1) and outproj(j-1) (PE-pure / mixed) so that every
            # engine has ready work throughout the chunk.

            def vproj_closures(st):
                st_ = {}

                def c1():
                    pv = ps.tile([P, OC], f32, tag="proj", bufs=2, name="pv")
                    st_["pv"] = pv
                    for k in range(4):
                        nc.tensor.matmul(
                            pv[:, :], xt_sb[:, k, P * st:P * st + P],
                            wv_sb[:, k, :], start=(k == 0), stop=False)

                def c2():
                    pv = st_["pv"]
                    for k in range(4, NK):
                        nc.tensor.matmul(
                            pv[:, :], xt_sb[:, k, P * st:P * st + P],
                            wv_sb[:, k, :], start=False, stop=(k == NK - 1))

                def c3():
                    pv4 = st_["pv"][:, :].rearrange("p (g x d) -> p g x d",
                                                    g=2, x=2)
                    nc.scalar.copy(vv0[:, st, :, 0:64], pv4[:, :, 0, :])
                    nc.scalar.copy(vv1[:, st, :, 64:P], pv4[:, :, 1, :])

                return [c1, c2, c3]

            def pass1_closures(pr, t):
                # scores [sq, sk] for one sq-tile, 2-head tile-packed; fused
                # (+causal mask) -> rowmax into m2 columns.
                ncols = (t + 1) * P
                nch = (ncols + 511) // 512
                st_ = {}

                def mk(c):
                    def cl():
                        n = min(512, ncols - 512 * c)
                        sa = ps.tile([P, 512], f32, tag="s1", bufs=2, name="sa")
                        sb_ = ps.tile([P, 512], f32, tag="s1", bufs=2,
                                      name="sb_")
                        last = c == nch - 1
                        if c == 0 and nch > 1:
                            st_["m4a"] = tmp.tile([P, 4], f32, tag="m4a",
                                                  bufs=2, name="m4a")
                            st_["m4b"] = tmp.tile([P, 4], f32, tag="m4b",
                                                  bufs=2, name="m4b")
                        nc.tensor.matmul(
                            sa[:, :n], qp_sb[0:64, pr, P * t:P * t + P],
                            kp_sb[0:64, pr, 512 * c:512 * c + n],
                            start=True, stop=not last, tile_position=(0, 0))
                        nc.tensor.matmul(
                            sb_[:, :n], qp_sb[64:P, pr, P * t:P * t + P],
                            kp_sb[64:P, pr, 512 * c:512 * c + n],
                            start=True, stop=not last, tile_position=(64, 0))
                        if last:
                            doff = n - P
                            nc.tensor.matmul(sa[:, doff:doff + P],
                                             ident16[:, :], cmask16[:, :],
                                             start=False, stop=True)
                            nc.tensor.matmul(sb_[:, doff:doff + P],
                                             ident16[:, :], cmask16[:, :],
                                             start=False, stop=True)
                        if nch == 1:
                            nc.vector.reduce_max(m2[:, pr, t:t + 1], sa[:, :n],
                                                 axis=AX)
                            nc.vector.reduce_max(m2[:, pr, 16 + t:16 + t + 1],
                                                 sb_[:, :n], axis=AX)
                        else:
                            m4a, m4b = st_["m4a"], st_["m4b"]
                            nc.vector.reduce_max(m4a[:, c:c + 1], sa[:, :n],
                                                 axis=AX)
                            nc.vector.reduce_max(m4b[:, c:c + 1], sb_[:, :n],
                                                 axis=AX)
                            if last:
                                nc.vector.reduce_max(m2[:, pr, t:t + 1],
                                                     m4a[:, 0:nch], axis=AX)
                                nc.vector.reduce_max(
                                    m2[:, pr, 16 + t:16 + t + 1],
                                    m4b[:, 0:nch], axis=AX)
                    return cl

                return [mk(c) for c in range(nch)]

            def mrow_closure(pr, j):
                # -M for chunk j's four sq-tiles -> row 64 of qaug, per head.
                def cl():
                    for hh in range(2):
                        mt_ps = ps.tile([4, P], f32, tag="s1", bufs=2,
                                        name="mt_ps")
                        nc.tensor.transpose(
                            mt_ps[:, :],
                            m2[:, pr, 16 * hh + 4 * j:16 * hh + 4 * j + 4],
                            ident[:, :])
                        mt_t = tmp.tile([4, P], f16, tag="mt", bufs=2,
                                        name="mt_t")
                        nc.scalar.activation(mt_t[:, :], mt_ps[:, :], COPY,
                                             scale=-1.0)
                        nc.sync.dma_start(
                            out=qaug[64:65, 2 * pr + hh,
                                     512 * j:512 * j + 512].rearrange(
                                         "q (t p) -> q t p", t=4),
                            in_=mt_t[:, :])
                return [cl]

            def pass2_closures(h, j):
                # scoresT with folded -M (and -30000 below-diagonal mask
                # accumulated on the PE for diagonal tiles, so exp gives
                # exact zeros there with no cross-engine select), exp into
                # fp16, and the ctx matmul skewed two tiles behind its exp
                # so the PE never waits on the ACT pipeline tail.
                pr, hh = divmod(h, 2)
                nt_here = 4 * j + 4
                st_ = {"ctxq": []}

                def emit_ctx(force=False):
                    depth = 0 if force else 3
                    while len(st_["ctxq"]) > depth:
                        t, qoff, n, pt = st_["ctxq"].pop(0)
                        lhsT = (vv0[:, t, pr, :] if hh == 0
                                else vv1[:, t, pr, :])
                        nc.tensor.matmul(
                            st_["ctxp"][0:(65 if hh == 0 else P),
                                        qoff - 512 * j:qoff - 512 * j + n],
                            lhsT, pt[:, :n],
                            start=(t == 0), stop=(t == nt_here - 1))

                def mk(t):
                    def cl():
                        if t == 0:
                            st_["ctxp"] = ps.tile([P, 512], f32, tag="ctx",
                                                  bufs=2, name="ctxp")
                        if t < 4 * j:
                            qoff, n = 512 * j, 512
                        else:
                            qoff = P * t
                            n = 512 * j + 512 - P * t
                        s2p = ps.tile([P, 512], f32, tag="s2", bufs=2,
                                      name="s2p")
                        nc.tensor.matmul(
                            s2p[:, :n], kaug[:, h, P * t:P * t + P],
                            qaug[:, h, qoff:qoff + n], start=True, stop=True)
                        pt = strm.tile([P, 512], f16, tag="pt", bufs=8,
                                       name="pt")
                        nc.scalar.activation(pt[:, :n], s2p[:, :n], EXP,
                                             scale=8.0)
                        if t >= 4 * j:
                            # zero strictly-upper block at the diagonal (the
                            # ctx skew hides the Pool round-trip)
                            nc.gpsimd.affine_select(
                                out=pt[:, 0:P], in_=pt[:, 0:P],
                                compare_op=ALU.is_ge, fill=0.0, base=0,
                                pattern=[[1, P]], channel_multiplier=-1)
                        st_["ctxq"].append((t, qoff, n, pt))
                        emit_ctx()
                    return cl

                def norm():
                    emit_ctx(force=True)
                    # normalize: ctx / rowsum
                    ctxp = st_["ctxp"]
                    rsrow = 64 if hh == 0 else 0
                    rr = tmp.tile([65, 512], f32, tag="rr", bufs=2, name="rr")
                    nc.vector.reciprocal(rr[rsrow:rsrow + 1, :],
                                         ctxp[rsrow:rsrow + 1, :])
                    rb = tmp.tile([P, 512], f32, tag="rb", bufs=2, name="rb")
                    nc.sync.dma_start(
                        out=rb[64 * hh:64 * hh + 64, :],
                        in_=rr[rsrow:rsrow + 1, :].unsqueeze(1).broadcast_to(
                            (1, 64, 512)))
                    nc.vector.tensor_mul(
                        ctxn[64 * hh:64 * hh + 64, pr, 512 * j:512 * j + 512],
                        ctxp[64 * hh:64 * hh + 64, :],
                        rb[64 * hh:64 * hh + 64, :])

                return [mk(t) for t in range(nt_here)] + [norm]

            outT_v = outT[:].rearrange("(o p) s -> p o s", p=P)

            def outproj_closures(j, copy_dve=False):
                st_ = {}

                def mk(oo):
                    def cl():
                        if oo == 0:
                            st_["ob"] = strm.tile([P, NK, 512], f16, tag="ob",
                                                  bufs=2, name="ob")
                        ob = st_["ob"]
                        po = ps.tile([P, 512], f32, tag="proj", bufs=2,
                                     name="po")
                        for kt in range(2):
                            nc.tensor.matmul(
                                po[:, :], wo_sb[:, kt, P * oo:P * oo + P],
                                ctxn[:, kt, 512 * j:512 * j + 512],
                                start=(kt == 0), stop=(kt == 1))
                        if copy_dve:
                            nc.vector.tensor_copy(ob[:, oo, :], po[:, :])
                        else:
                            nc.scalar.copy(ob[:, oo, :], po[:, :])
                        if oo == 3:
                            nc.sync.dma_start(
                                out=outT_v[:, 0:4, 512 * j:512 * j + 512],
                                in_=ob[:, 0:4, :])
                        elif oo == 7:
                            nc.sync.dma_start(
                                out=outT_v[:, 4:NK, 512 * j:512 * j + 512],
                                in_=ob[:, 4:NK, :])
                    return cl

                return [mk(oo) for oo in range(E // P)]

            def merge_emit(a_ops, b_ops, lead_b=5):
                # Proportionally interleave b_ops into a_ops (at most one
                # b per a so dep-blocked matmuls never pile up past the
                # 4-deep engine wait queue); all b_ops are drained before
                # returning, which callers rely on for cross-stream deps.
                bq = list(b_ops)
                if not a_ops:
                    for b in bq:
                        b()
                    return
                for _ in range(min(lead_b, len(bq))):
                    bq.pop(0)()
                ratio = len(bq) / len(a_ops)
                acc = 0.0
                for a in a_ops:
                    a()
                    acc += ratio
                    if acc >= 1.0 and bq:
                        bq.pop(0)()
                        acc -= 1.0
                for b in bq:
                    b()

            # ---- software-pipelined attention, seq chunks processed in
            # DESCENDING order ----
            # The DVE row-max reduce train is the critical path; each chunk's
            # softmax (ACT exp) can only start after its own reduces, so the
            # train runs largest-consumer-first (j=3 ... 0) and the last
            # window leaves only the cheapest exp work + one out-proj after
            # the train drains (flow-shop tail minimization).
            for cl in kproj_wave_closures(0, ["s1", "s1", "s2", "s2"]):
                cl()
            for cl in qaug_closure():
                cl()
            a_lead = []
            for t in range(12, 16):
                a_lead += pass1_closures(0, t)
            a_lead += mrow_closure(0, 3)
            b_lead = kproj_wave_closures(1, ["ctx", "ctx", "proj", "proj"])
            for st in range(NT):
                b_lead += vproj_closures(st)
            merge_emit(a_lead, b_lead)
            for j in (3, 2, 1, 0):
                # pr0 heads merge against [pass1(pr1, j) + mrow(pr1, j)]
                # (fully drained before the pr1 heads' pass2 reads the -M
                # row), pr1 heads against the next train segment.
                a0 = pass2_closures(0, j) + pass2_closures(1, j)
                a1 = pass2_closures(2, j) + pass2_closures(3, j)
                b0 = []
                if j < 3:
                    b0 += outproj_closures(j + 1)
                for t in range(4 * j, 4 * j + 4):
                    b0 += pass1_closures(1, t)
                b0 += mrow_closure(1, j)
                b1 = []
                if j > 0:
                    for t in range(4 * (j - 1), 4 * (j - 1) + 4):
                        b1 += pass1_closures(0, t)
                    b1 += mrow_closure(0, j - 1)
                half = len(b1) // 2
                merge_emit(a0, b0 + b1[:half])
                merge_emit(a1, b1[half:])
            for cl in outproj_closures(0, copy_dve=True):
                cl()

    nc.compile()
    return nc


def _get_nc():
    if "nc" not in _CACHE:
        _CACHE["nc"] = _build()
    return _CACHE["nc"]


def _make_cached_runner(nc):
    """Trace/compile the 8-core PJRT executable once; reuse on later calls.

    Mirrors concourse.bass2jax.run_bass_via_pjrt's multi-core branch, but
    keeps the jitted shard_map so repeat kernel() calls skip re-trace and
    re-lowering (the NEFF itself is already cached by neuronx_cc_hook).
    """
    import jax
    import jax.numpy as jnp
    from jax.sharding import Mesh, PartitionSpec
    from jax.experimental.shard_map import shard_map
    from concourse import bass2jax, mybir

    bass2jax.install_neuronx_cc_hook()
    partition_name = nc.partition_id_tensor.name if nc.partition_id_tensor else None
    in_names, out_names, out_avals = [], [], []
    for alloc in nc.m.functions[0].allocations:
        if not isinstance(alloc, mybir.MemoryLocationSet):
            continue
        name = alloc.memorylocations[0].name
        if alloc.kind == "ExternalInput":
            if name != partition_name:
                in_names.append(name)
        elif alloc.kind == "ExternalOutput":
            out_names.append(name)
            out_avals.append(jax.core.ShapedArray(
                tuple(alloc.tensor_shape), mybir.dt.np(alloc.dtype)))
    n_params = len(in_names)
    n_outs = len(out_avals)
    all_names = list(in_names) + list(out_names)
    if partition_name is not None:
        all_names.append(partition_name)

    def _body(*args):
        operands = list(args)
        if partition_name is not None:
            operands.append(bass2jax.partition_id_tensor())
        outs = bass2jax._bass_exec_p.bind(
            *operands,
            out_avals=tuple(out_avals),
            in_names=tuple(all_names),
            out_names=tuple(out_names),
            lowering_input_output_aliases=(),
            sim_require_finite=True,
            sim_require_nnan=True,
            nc=nc,
        )
        return tuple(outs)

    devices = jax.devices()[:NCORES]
    mesh = Mesh(np.asarray(devices), ("core",))
    in_specs = (PartitionSpec("core"),) * (n_params + n_outs)
    out_specs = (PartitionSpec("core"),) * n_outs
    sharded = jax.jit(
        shard_map(_body, mesh=mesh, in_specs=in_specs, out_specs=out_specs,
                  check_rep=False),
        donate_argnums=tuple(range(n_params, n_params + n_outs)),
        keep_unused=True)

    def run(in_maps):
        concat_in = [
            np.concatenate([np.asarray(in_maps[c][nm]) for c in range(NCORES)],
                           axis=0)
            for nm in in_names]
        concat_zeros = [
            np.zeros((NCORES * a.shape[0], *a.shape[1:]), a.dtype)
            for a in out_avals]
        out_arrs = sharded(*concat_in, *concat_zeros)
        return [
            {nm: np.asarray(out_arrs[i]).reshape(NCORES, *out_avals[i].shape)[c]
             for i, nm in enumerate(out_names)}
            for c in range(NCORES)]

    return run


def kernel(x, Wq, Wk, Wv, Wo):
    from concourse.bass_utils import run_bass_kernel_spmd

    # Force host numpy immediately: if the caller hands us jax arrays, any
    # .astype/.T on them would dispatch tiny jit programs to the neuron
    # backend, which wedges the device (known neuron-jit crash path).
    x, Wq, Wk, Wv, Wo = (np.asarray(a) for a in (x, Wq, Wk, Wv, Wo))

    nc = _get_nc()
    x16 = np.ascontiguousarray(x.astype(np.float16))
    Wq16 = Wq.astype(np.float16)
    Wk16 = Wk.astype(np.float16)
    Wv16 = Wv.astype(np.float16)
    Wo16 = Wo.astype(np.float16)

    xTs = [np.ascontiguousarray(x16[b].T) for b in range(B)]
    in_maps = []
    for c in range(NCORES):
        b, hg = divmod(c, HG)
        hsl = slice(OC * hg, OC * hg + OC)
        in_maps.append({
            "xt": xTs[b],
            "wq": np.ascontiguousarray(Wq16[hsl, :].T),
            "wk": np.ascontiguousarray(Wk16[hsl, :].T),
            "wv": np.ascontiguousarray(Wv16[hsl, :].T),
            "wo": np.ascontiguousarray(Wo16[:, hsl].T),
        })

    if "runner" in _CACHE:
        results = _CACHE["runner"](in_maps)
    else:
        # first call: compile + run through the sanctioned entry point,
        # then build the cached executable for subsequent calls
        results = run_bass_kernel_spmd(nc, in_maps, list(range(NCORES))).results
        _CACHE["runner"] = _make_cached_runner(nc)

    out = np.zeros((B, S, E), np.float32)
    for c in range(NCORES):
        b = c // HG
        out[b] += results[c]["outT"].T.astype(np.float32)
    return out


# revision 20
# speedup vs baseline: 1.2157x; 1.0276x over previous
"""Trainium2 Bass kernel for nn_AutoregressiveSelfAttention.

Sharding (8 cores): batch (2-way) x head-group (4-way tensor parallel).
Core c: batch c//4, heads [4*(c%4), 4*(c%4)+4).
Per-core: fp16 matmuls throughout (QKV proj, scores, P@V, out-proj),
fp32 softmax statistics, fp16 partial output; host sums the 4 head-group
partials per batch (the row-parallel all-reduce) in fp32 and transposes.

Softmax without transposes: pass1 computes scores [sq, sk] only to get the
causal row-max M (fused mask+max on DVE); pass2 recomputes scores
transposed with the max-subtraction folded in as a rank-1 matmul term
([kT;1].T @ [qT;-M]), exps on ACT into fp16, and the ctx matmul against
[v|1] accumulates both ctx and the softmax denominator in one PSUM tile.

v2 restructure vs baseline:
- q/k projections run contraction-outer across all 8 PSUM banks so the
  first matmuls start as soon as the first xt chunk lands (DMA-paced).
- Input DMAs batched per tensor (wq, wk, xt x8, wv, wo); augmented q/k
  copies batched to one [64, S] DMA per (tensor, head) issued upfront.
- Engine rebalance: projection PSUM->SBUF copies and out-proj copies on
  ACT (was DVE), -M negate folded into an ACT copy (scale=-1), DVE keeps
  only the row-max reduces, reciprocals and normalize multiplies.
- Emission order software-pipelines across chunks: pass1(pr1) hides
  mrow(pr0) DMA latency, vproj(j+1) hides normalize latency before
  outproj(j).
- Output stored fp16 and written with one batched DMA per seq chunk.
"""
import sys
sys.path.insert(0, "/opt/trn_rl_repo")

import math
import numpy as np

B, S, E, H, D = 2, 2048, 1024, 16, 64
NCORES = 8
HG = 4                  # head-group shards
HPC = H // HG           # 4 heads per core
OC = HPC * D            # 256 per-core projection width
P = 128
NK = E // P             # 8 contraction tiles for projections
NT = S // P             # 16 seq tiles of 128
NJ = S // 512           # 4 seq chunks of 512

_CACHE = {}


def _build():
    import concourse.bacc as bacc
    import concourse.mybir as mybir
    import concourse.tile as tile
    from concourse.masks import make_identity, make_causal_mask

    dt = mybir.dt
    f32, f16 = dt.float32, dt.float16
    AX = mybir.AxisListType.X
    ALU = mybir.AluOpType
    COPY = mybir.ActivationFunctionType.Copy
    EXP = mybir.ActivationFunctionType.Exp

    nc = bacc.Bacc(None, target_bir_lowering=False, debug=False)
    with tile.TileContext(nc) as tc:
        with tc.tile_pool(name="dram", bufs=1, space="DRAM") as dram, \
             tc.tile_pool(name="persist", bufs=1) as pers, \
             tc.tile_pool(name="stream", bufs=4) as strm, \
             tc.tile_pool(name="tmp", bufs=4) as tmp, \
             tc.tile_pool(name="ps", bufs=1, space="PSUM") as ps:

            # ---- DRAM I/O ----
            xt = dram.tile([E, S], f16, kind="ExternalInput", name="xt", uniquify=False)
            wq = dram.tile([E, OC], f16, kind="ExternalInput", name="wq", uniquify=False)
            wk = dram.tile([E, OC], f16, kind="ExternalInput", name="wk", uniquify=False)
            wv = dram.tile([E, OC], f16, kind="ExternalInput", name="wv", uniquify=False)
            wo = dram.tile([OC, E], f16, kind="ExternalInput", name="wo", uniquify=False)
            outT = dram.tile([E, S], f16, kind="ExternalOutput", name="outT", uniquify=False)

            # ---- persistent SBUF ----
            xt_sb = pers.tile([P, NK, S], f16)
            wq_sb = pers.tile([P, NK, OC], f16)
            wk_sb = pers.tile([P, NK, OC], f16)
            wv_sb = pers.tile([P, NK, OC], f16)
            wo_sb = pers.tile([P, 2, E], f16)
            qp_sb = pers.tile([P, 2, S], f16)       # qT, head-pair stacked
            kp_sb = pers.tile([P, 2, S], f16)       # kT, head-pair stacked
            qaug = pers.tile([65, HPC, S], f16)     # [qT_h ; -M_h] per head
            kaug = pers.tile([65, HPC, S], f16)     # [kT_h ; ones] per head
            vv0 = pers.tile([P, NT, 2, 65], f16)    # heads 0,2: [v(0:64) | ones]
            vv1 = pers.tile([P, NT, 2, P], f16)     # heads 1,3: [ones|0*63|v(64:128)]
            ctxn = pers.tile([P, 2, S], f16)        # normalized ctx, pair stacked
            m2 = pers.tile([P, 2, 32], f32)         # rowmax cols per pair (hh*16+t)
            ident = pers.tile([P, P], f32)
            ident16 = pers.tile([P, P], f16)
            cmask16 = pers.tile([P, P], f16)        # 0 / -30000 above diag
            cml16 = pers.tile([P, P], f16)          # 0 / -30000 below diag

            # ---- input DMAs (batched, consumer order) ----
            xt_v = xt[:].rearrange("(k p) s -> p k s", p=P)
            nc.sync.dma_start(out=wq_sb[:, :, :],
                              in_=wq[:].rearrange("(k p) o -> p k o", p=P))
            for k in range(NK):
                nc.sync.dma_start(out=xt_sb[:, k, :], in_=xt_v[:, k, :])
            nc.sync.dma_start(out=wk_sb[:, :, :],
                              in_=wk[:].rearrange("(k p) o -> p k o", p=P))
            nc.sync.dma_start(out=wv_sb[:, :, :],
                              in_=wv[:].rearrange("(k p) o -> p k o", p=P))
            nc.sync.dma_start(out=wo_sb[:, :, :],
                              in_=wo[:].rearrange("(k p) e -> p k e", p=P))

            # ---- constants ----
            make_identity(nc, ident[:, :])
            make_identity(nc, ident16[:, :])
            make_causal_mask(nc, cmask16[:, :], mask_val=-30000.0)
            # cml16: -30000 strictly below the diagonal (masks k > q in the
            # transposed scores), built by affine-filling a zero tile.
            nc.gpsimd.memset(cml16[:, :], 0.0)
            nc.gpsimd.affine_select(
                out=cml16[:, :], in_=cml16[:, :],
                compare_op=ALU.is_ge, fill=-30000.0, base=0,
                pattern=[[1, P]], channel_multiplier=-1)
            nc.gpsimd.memset(kaug[64:65, :, :], 1.0)
            nc.gpsimd.memset(vv0[:, :, :, 64:65], 1.0)
            nc.gpsimd.memset(vv1[:, :, :, 0:1], 1.0)
            nc.gpsimd.memset(vv1[:, :, :, 1:64], 0.0)

            # ---- q projection: contraction-outer over all 8 PSUM banks so
            # matmuls pace with the xt chunk DMAs ----
            TAGS8 = ["s1", "s1", "s2", "s2", "ctx", "ctx", "proj", "proj"]
            pp = []
            for i in range(8):
                t_ = ps.tile([P, 512], f32, tag=TAGS8[i], bufs=2,
                             name=f"pp{i}")
                pp.append(t_)
            for k in range(NK):
                for i in range(8):
                    ot, j = divmod(i, NJ)
                    nc.tensor.matmul(
                        pp[i][:, :],
                        wq_sb[:, k, 128 * ot:128 * ot + 128],
                        xt_sb[:, k, 512 * j:512 * j + 512],
                        start=(k == 0), stop=(k == NK - 1))
            for i in range(8):
                ot, j = divmod(i, NJ)
                sl = slice(512 * j, 512 * j + 512)
                if i % 2 == 0:
                    nc.vector.tensor_copy(qp_sb[:, ot, sl], pp[i][:, :])
                else:
                    nc.scalar.copy(qp_sb[:, ot, sl], pp[i][:, :])

            def kproj_wave_closures(ot, tags):
                # one head-pair of the k projection: 4 tiles, k-outer
                st_ = {}

                def alloc():
                    st_["kp"] = [ps.tile([P, 512], f32, tag=tags[i], bufs=2,
                                         name=f"kpp{i}") for i in range(4)]

                def mmk(k):
                    def cl():
                        if k == 0:
                            alloc()
                        for jj in range(NJ):
                            nc.tensor.matmul(
                                st_["kp"][jj][:, :],
                                wk_sb[:, k, 128 * ot:128 * ot + 128],
                                xt_sb[:, k, 512 * jj:512 * jj + 512],
                                start=(k == 0), stop=(k == NK - 1))
                    return cl

                def cp():
                    for jj in range(NJ):
                        sl = slice(512 * jj, 512 * jj + 512)
                        if jj % 2 == 0:
                            nc.vector.tensor_copy(kp_sb[:, ot, sl],
                                                  st_["kp"][jj][:, :])
                        else:
                            nc.scalar.copy(kp_sb[:, ot, sl],
                                           st_["kp"][jj][:, :])
                    for hh in range(2):
                        h = 2 * ot + hh
                        nc.sync.dma_start(
                            out=kaug[0:64, h, :],
                            in_=kp_sb[64 * hh:64 * hh + 64, ot, :])
                return [mmk(k) for k in range(NK)] + [cp]

            def qaug_closure():
                def cl():
                    for h in range(HPC):
                        pr, hh = divmod(h, 2)
                        nc.sync.dma_start(
                            out=qaug[0:64, h, :],
                            in_=qp_sb[64 * hh:64 * hh + 64, pr, :])
                return [cl]

            # Emission below is organized as lists of closures ("streams")
            # that are proportionally interleaved: pass2(j) (ACT-paced) is
            # merged instruction-by-instruction with pass1(j+1) (DVE-paced),
            # vproj(j+1) and outproj(j-1) (PE-pure / mixed) so that every
            # engine has ready work throughout the chunk.

            def vproj_closures(st):
                st_ = {}

                def c1():
                    pv = ps.tile([P, OC], f32, tag="proj", bufs=2, name="pv")
                    st_["pv"] = pv
                    for k in range(4):
                        nc.tensor.matmul(
                            pv[:, :], xt_sb[:, k, P * st:P * st + P],
                            wv_sb[:, k, :], start=(k == 0), stop=False)

                def c2():
                    pv = st_["pv"]
                    for k in range(4, NK):
                        nc.tensor.matmul(
                            pv[:, :], xt_sb[:, k, P * st:P * st + P],
                            wv_sb[:, k, :], start=False, stop=(k == NK - 1))

                def c3():
                    pv4 = st_["pv"][:, :].rearrange("p (g x d) -> p g x d",
                                                    g=2, x=2)
                    nc.scalar.copy(vv0[:, st, :, 0:64], pv4[:, :, 0, :])
                    nc.scalar.copy(vv1[:, st, :, 64:P], pv4[:, :, 1, :])

                return [c1, c2, c3]

            def pass1_closures(pr, t):
                # scores [sq, sk] for one sq-tile, 2-head tile-packed; fused
                # (+causal mask) -> rowmax into m2 columns.
                ncols = (t + 1) * P
                nch = (ncols + 511) // 512
                st_ = {}

                def mk(c):
                    def cl():
                        n = min(512, ncols - 512 * c)
                        sa = ps.tile([P, 512], f32, tag="s1", bufs=2, name="sa")
                        sb_ = ps.tile([P, 512], f32, tag="s1", bufs=2,
                                      name="sb_")
                        last = c == nch - 1
                        if c == 0 and nch > 1:
                            st_["m4a"] = tmp.tile([P, 4], f32, tag="m4a",
                                                  bufs=2, name="m4a")
                            st_["m4b"] = tmp.tile([P, 4], f32, tag="m4b",
                                                  bufs=2, name="m4b")
                        nc.tensor.matmul(
                            sa[:, :n], qp_sb[0:64, pr, P * t:P * t + P],
                            kp_sb[0:64, pr, 512 * c:512 * c + n],
                            start=True, stop=not last, tile_position=(0, 0))
                        nc.tensor.matmul(
                            sb_[:, :n], qp_sb[64:P, pr, P * t:P * t + P],
                            kp_sb[64:P, pr, 512 * c:512 * c + n],
                            start=True, stop=not last, tile_position=(64, 0))
                        if last:
                            doff = n - P
                            nc.tensor.matmul(sa[:, doff:doff + P],
                                             ident16[:, :], cmask16[:, :],
                                             start=False, stop=True)
                            nc.tensor.matmul(sb_[:, doff:doff + P],
                                             ident16[:, :], cmask16[:, :],
                                             start=False, stop=True)
                        if nch == 1:
                            nc.vector.reduce_max(m2[:, pr, t:t + 1], sa[:, :n],
                                                 axis=AX)
                            nc.vector.reduce_max(m2[:, pr, 16 + t:16 + t + 1],
                                                 sb_[:, :n], axis=AX)
                        else:
                            m4a, m4b = st_["m4a"], st_["m4b"]
                            nc.vector.reduce_max(m4a[:, c:c + 1], sa[:, :n],
                                                 axis=AX)
                            nc.vector.reduce_max(m4b[:, c:c + 1], sb_[:, :n],
                                                 axis=AX)
                            if last:
                                nc.vector.reduce_max(m2[:, pr, t:t + 1],
                                                     m4a[:, 0:nch], axis=AX)
                                nc.vector.reduce_max(
                                    m2[:, pr, 16 + t:16 + t + 1],
                                    m4b[:, 0:nch], axis=AX)
                    return cl

                return [mk(c) for c in range(nch)]

            def mrow_closure(pr, j):
                # -M for chunk j's four sq-tiles -> row 64 of qaug, per head.
                def cl():
                    for hh in range(2):
                        mt_ps = ps.tile([4, P], f32, tag="s1", bufs=2,
                                        name="mt_ps")
                        nc.tensor.transpose(
                            mt_ps[:, :],
                            m2[:, pr, 16 * hh + 4 * j:16 * hh + 4 * j + 4],
                            ident[:, :])
                        mt_t = tmp.tile([4, P], f16, tag="mt", bufs=2,
                                        name="mt_t")
                        nc.scalar.activation(mt_t[:, :], mt_ps[:, :], COPY,
                                             scale=-1.0)
                        nc.sync.dma_start(
                            out=qaug[64:65, 2 * pr + hh,
                                     512 * j:512 * j + 512].rearrange(
                                         "q (t p) -> q t p", t=4),
                            in_=mt_t[:, :])
                return [cl]

            def pass2_closures(h, j):
                # scoresT with folded -M (and -30000 below-diagonal mask
                # accumulated on the PE for diagonal tiles, so exp gives
                # exact zeros there with no cross-engine select), exp into
                # fp16, and the ctx matmul skewed two tiles behind its exp
                # so the PE never waits on the ACT pipeline tail.
                pr, hh = divmod(h, 2)
                nt_here = 4 * j + 4
                st_ = {"ctxq": []}

                def emit_ctx(force=False):
                    depth = 0 if force else 3
                    while len(st_["ctxq"]) > depth:
                        t, qoff, n, pt = st_["ctxq"].pop(0)
                        lhsT = (vv0[:, t, pr, :] if hh == 0
                                else vv1[:, t, pr, :])
                        nc.tensor.matmul(
                            st_["ctxp"][0:(65 if hh == 0 else P),
                                        qoff - 512 * j:qoff - 512 * j + n],
                            lhsT, pt[:, :n],
                            start=(t == 0), stop=(t == nt_here - 1))

                def mk(t):
                    def cl():
                        if t == 0:
                            st_["ctxp"] = ps.tile([P, 512], f32, tag="ctx",
                                                  bufs=2, name="ctxp")
                        if t < 4 * j:
                            qoff, n = 512 * j, 512
                        else:
                            qoff = P * t
                            n = 512 * j + 512 - P * t
                        s2p = ps.tile([P, 512], f32, tag="s2", bufs=2,
                                      name="s2p")
                        nc.tensor.matmul(
                            s2p[:, :n], kaug[:, h, P * t:P * t + P],
                            qaug[:, h, qoff:qoff + n], start=True, stop=True)
                        pt = strm.tile([P, 512], f16, tag="pt", bufs=8,
                                       name="pt")
                        nc.scalar.activation(pt[:, :n], s2p[:, :n], EXP,
                                             scale=8.0)
                        if t >= 4 * j:
                            # zero strictly-upper block at the diagonal (the
                            # ctx skew hides the Pool round-trip)
                            nc.gpsimd.affine_select(
                                out=pt[:, 0:P], in_=pt[:, 0:P],
                                compare_op=ALU.is_ge, fill=0.0, base=0,
                                pattern=[[1, P]], channel_multiplier=-1)
                        st_["ctxq"].append((t, qoff, n, pt))
                        emit_ctx()
                    return cl

                def norm():
                    emit_ctx(force=True)
                    # normalize: ctx / rowsum
                    ctxp = st_["ctxp"]
                    rsrow = 64 if hh == 0 else 0
                    rr = tmp.tile([65, 512], f32, tag="rr", bufs=2, name="rr")
                    nc.vector.reciprocal(rr[rsrow:rsrow + 1, :],
                                         ctxp[rsrow:rsrow + 1, :])
                    rb = tmp.tile([P, 512], f32, tag="rb", bufs=2, name="rb")
                    nc.sync.dma_start(
                        out=rb[64 * hh:64 * hh + 64, :],
                        in_=rr[rsrow:rsrow + 1, :].unsqueeze(1).broadcast_to(
                            (1, 64, 512)))
                    nc.vector.tensor_mul(
                        ctxn[64 * hh:64 * hh + 64, pr, 512 * j:512 * j + 512],
                        ctxp[64 * hh:64 * hh + 64, :],
                        rb[64 * hh:64 * hh + 64, :])

                return [mk(t) for t in range(nt_here)] + [norm]

            outT_v = outT[:].rearrange("(o p) s -> p o s", p=P)

            def outproj_closures(j, copy_dve=False):
                st_ = {}

                def mk(oo):
                    def cl():
                        if oo == 0:
                            st_["ob"] = strm.tile([P, NK, 512], f16, tag="ob",
                                                  bufs=2, name="ob")
                        ob = st_["ob"]
                        po = ps.tile([P, 512], f32, tag="proj", bufs=2,
                                     name="po")
                        for kt in range(2):
                            nc.tensor.matmul(
                                po[:, :], wo_sb[:, kt, P * oo:P * oo + P],
                                ctxn[:, kt, 512 * j:512 * j + 512],
                                start=(kt == 0), stop=(kt == 1))
                        if copy_dve:
                            nc.vector.tensor_copy(ob[:, oo, :], po[:, :])
                        else:
                            nc.scalar.copy(ob[:, oo, :], po[:, :])
                        if oo % 2 == 1:
                            nc.sync.dma_start(
                                out=outT_v[:, oo - 1:oo + 1,
                                           512 * j:512 * j + 512],
                                in_=ob[:, oo - 1:oo + 1, :])
                    return cl

                return [mk(oo) for oo in range(E // P)]

            def merge_emit(a_ops, b_ops, lead_b=5):
                # Proportionally interleave b_ops into a_ops (at most one
                # b per a so dep-blocked matmuls never pile up past the
                # 4-deep engine wait queue); all b_ops are drained before
                # returning, which callers rely on for cross-stream deps.
                bq = list(b_ops)
                if not a_ops:
                    for b in bq:
                        b()
                    return
                for _ in range(min(lead_b, len(bq))):
                    bq.pop(0)()
                ratio = len(bq) / len(a_ops)
                acc = 0.0
                for a in a_ops:
                    a()
                    acc += ratio
                    if acc >= 1.0 and bq:
                        bq.pop(0)()
                        acc -= 1.0
                for b in bq:
                    b()

            # ---- software-pipelined attention, seq chunks processed in
            # DESCENDING order ----
            # The DVE row-max reduce train is the critical path; each chunk's
            # softmax (ACT exp) can only start after its own reduces, so the
            # train runs largest-consumer-first (j=3 ... 0) and the last
            # window leaves only the cheapest exp work + one out-proj after
            # the train drains (flow-shop tail minimization).
            for cl in kproj_wave_closures(0, ["s1", "s1", "s2", "s2"]):
                cl()
            for cl in qaug_closure():
                cl()
            a_lead = []
            for t in range(12, 16):
                a_lead += pass1_closures(0, t)
            a_lead += mrow_closure(0, 3)
            b_lead = kproj_wave_closures(1, ["ctx", "ctx", "proj", "proj"])
            for st in range(NT):
                b_lead += vproj_closures(st)
            merge_emit(a_lead, b_lead)
            for j in (3, 2, 1, 0):
                # pr0 heads merge against [pass1(pr1, j) + mrow(pr1, j)]
                # (fully drained before the pr1 heads' pass2 reads the -M
                # row), pr1 heads against the next train segment.
                a0 = pass2_closures(0, j) + pass2_closures(1, j)
                a1 = pass2_closures(2, j) + pass2_closures(3, j)
                # ready pass1 work leads b0 so dep-blocked outproj matmuls
                # (waiting on the previous window's last normalize) never
                # jam the PE wait queue at the window boundary
                b0 = []
                for t in range(4 * j, 4 * j + 4):
                    b0 += pass1_closures(1, t)
                b0 += mrow_closure(1, j)
                if j < 3:
                    b0 += outproj_closures(j + 1)
                b1 = []
                if j > 0:
                    for t in range(4 * (j - 1), 4 * (j - 1) + 4):
                        b1 += pass1_closures(0, t)
                    b1 += mrow_closure(0, j - 1)
                half = len(b1) // 2
                merge_emit(a0, b0 + b1[:half])
                merge_emit(a1, b1[half:])
            for cl in outproj_closures(0, copy_dve=True):
                cl()

    nc.compile()
    return nc


def _get_nc():
    if "nc" not in _CACHE:
        _CACHE["nc"] = _build()
    return _CACHE["nc"]


def _make_cached_runner(nc):
    """Trace/compile the 8-core PJRT executable once; reuse on later calls.

    Mirrors concourse.bass2jax.run_bass_via_pjrt's multi-core branch, but
    keeps the jitted shard_map so repeat kernel() calls skip re-trace and
    re-lowering (the NEFF itself is already cached by neuronx_cc_hook).
    """
    import jax
    import jax.numpy as jnp
    from jax.sharding import Mesh, PartitionSpec
    from jax.experimental.shard_map import shard_map
    from concourse import bass2jax, mybir

    bass2jax.install_neuronx_cc_hook()
    partition_name = nc.partition_id_tensor.name if nc.partition_id_tensor else None
    in_names, out_names, out_avals = [], [], []
    for alloc in nc.m.functions[0].allocations:
        if not isinstance(alloc, mybir.MemoryLocationSet):
            continue
        name = alloc.memorylocations[0].name
        if alloc.kind == "ExternalInput":
            if name != partition_name:
                in_names.append(name)
        elif alloc.kind == "ExternalOutput":
            out_names.append(name)
            out_avals.append(jax.core.ShapedArray(
                tuple(alloc.tensor_shape), mybir.dt.np(alloc.dtype)))
    n_params = len(in_names)
    n_outs = len(out_avals)
    all_names = list(in_names) + list(out_names)
    if partition_name is not None:
        all_names.append(partition_name)

    def _body(*args):
        operands = list(args)
        if partition_name is not None:
            operands.append(bass2jax.partition_id_tensor())
        outs = bass2jax._bass_exec_p.bind(
            *operands,
            out_avals=tuple(out_avals),
            in_names=tuple(all_names),
            out_names=tuple(out_names),
            lowering_input_output_aliases=(),
            sim_require_finite=True,
            sim_require_nnan=True,
            nc=nc,
        )
        return tuple(outs)

    devices = jax.devices()[:NCORES]
    mesh = Mesh(np.asarray(devices), ("core",))
    in_specs = (PartitionSpec("core"),) * (n_params + n_outs)
    out_specs = (PartitionSpec("core"),) * n_outs
    sharded = jax.jit(
        shard_map(_body, mesh=mesh, in_specs=in_specs, out_specs=out_specs,
                  check_rep=False),
        donate_argnums=tuple(range(n_params, n_params + n_outs)),
        keep_unused=True)

    def run(in_maps):
        concat_in = [
            np.concatenate([np.asarray(in_maps[c][nm]) for c in range(NCORES)],
                           axis=0)
            for nm in in_names]
        concat_zeros = [
            np.zeros((NCORES * a.shape[0], *a.shape[1:]), a.dtype)
            for a in out_avals]
        out_arrs = sharded(*concat_in, *concat_zeros)
        return [
            {nm: np.asarray(out_arrs[i]).reshape(NCORES, *out_avals[i].shape)[c]
             for i, nm in enumerate(out_names)}
            for c in range(NCORES)]

    return run


def kernel(x, Wq, Wk, Wv, Wo):
    from concourse.bass_utils import run_bass_kernel_spmd

    # Force host numpy immediately: if the caller hands us jax arrays, any
    # .astype/.T on them would dispatch tiny jit programs to the neuron
    # backend, which wedges the device (known neuron-jit crash path).
    x, Wq, Wk, Wv, Wo = (np.asarray(a) for a in (x, Wq, Wk, Wv, Wo))

    nc = _get_nc()
    x16 = np.ascontiguousarray(x.astype(np.float16))
    Wq16 = Wq.astype(np.float16)
    Wk16 = Wk.astype(np.float16)
    Wv16 = Wv.astype(np.float16)
    Wo16 = Wo.astype(np.float16)

    xTs = [np.ascontiguousarray(x16[b].T) for b in range(B)]
    in_maps = []
    for c in range(NCORES):
        b, hg = divmod(c, HG)
        hsl = slice(OC * hg, OC * hg + OC)
        in_maps.append({
            "xt": xTs[b],
            "wq": np.ascontiguousarray(Wq16[hsl, :].T),
            "wk": np.ascontiguousarray(Wk16[hsl, :].T),
            "wv": np.ascontiguousarray(Wv16[hsl, :].T),
            "wo": np.ascontiguousarray(Wo16[:, hsl].T),
        })

    if "runner" in _CACHE:
        results = _CACHE["runner"](in_maps)
    else:
        # first call: compile + run through the sanctioned entry point,
        # then build the cached executable for subsequent calls
        results = run_bass_kernel_spmd(nc, in_maps, list(range(NCORES))).results
        _CACHE["runner"] = _make_cached_runner(nc)

    out = np.zeros((B, S, E), np.float32)
    for c in range(NCORES):
        b = c // HG
        out[b] += results[c]["outT"].T.astype(np.float32)
    return out
